# revision 17
# baseline (speedup 1.0000x reference)
"""Causal self-attention (weight-modulated) Trainium2 kernel, 8-core SPMD.

Reference semantics (B=2, T=2048, C=512, 8 heads, hd=64):
    v0  = x @ Wv.T + bv
    att = softmax(mask((v0h @ v0h^T) * w[key] / sqrt(hd)))
    y   = att @ (v0*w[row])h
    out = y @ Wp.T + bp

Sharding: core = (b, hp) with b = batch, hp = head pair (v0 dims
[128hp, 128hp+128)).  v2 design notes:
  - w[key] is folded into wvT = vT * wrep (GpSimd), so QK against wvT
    yields pre-scaled scores and va = DMA-transpose(wvT) needs no
    further scaling.
  - exp is split across ACT (exact, off-diag blocks) and DVE
    (Schraudolph bf16 bit-trick: i16 = S*A + bias, bitcast; the causal
    mask of diagonal blocks is fused via scalar_tensor_tensor with a
    bias/mask tile: masked lanes get +4000 -> bitcast ~1e-25 ~ 0).
  - QK row-tiled: the two heads' K=64 matmuls run concurrently on
    array row strips (tile_position (0,0)/(64,0)).
  - softmax denominator rides as a 65th ones-column in the AV
    stationary; 1/d via bf16 magic-seed + 1 Newton step on DVE; the
    (negated) reciprocal is partition-broadcast by GpSimd and the sign
    is folded into a host-negated Wp.
  - PSUM is hand-placed in one [128, 4096] arena: banks 0-3 = S
    ping/pong, banks 4-7 = two yps pairs (alternating qj parity), with
    V-proj and out-proj matmuls reusing freed yps banks.
Host: out[b] = sum_hp outT^T + bp  (partial-sum reduce off-device).
"""

import ml_dtypes
import numpy as np

B, T, C = 2, 2048, 512
NH, HD = 8, 64
P = 128
QB = 512                 # query chunk
NQ = 4                   # query chunks
NKB = 16                 # key blocks of 128

EXP_A = 128 * 1.4426950408889634     # ln->bf16-exponent scale
EXP_BIAS = 16250.625                 # 16256 - C (C calibrated)
MASK_BIAS = 4000.0                   # masked lanes -> tiny positive
MAGIC16 = 0x7EF3                     # bf16 reciprocal seed

_cache = {}


def _split_multi_waits(nc, mybir):
    """Walrus in this container encodes at most ONE sync wait (and one
    update) per instruction; Tile's sem assignment emits several. Hoist
    excess waits onto single-wait NOPs placed just before the
    instruction on the same engine, and excess updates of non-DMA
    instructions onto NOPs just after."""
    dma_ops = {"DMACopy", "DMATranspose", "TensorCopy"}
    for f in nc.m.functions:
        for bb in f.blocks:
            new = []
            changed = False
            for inst in bb.instructions:
                si = inst.sync_info
                waits = list(si.on_wait or []) if si is not None else []
                ups = list(si.on_update or []) if si is not None else []
                is_dma = inst.concise_opcode() in dma_ops if hasattr(
                    inst, "concise_opcode") else False
                post = []
                if si is not None and len(waits) > 1:
                    for w in waits[:-1]:
                        nop = mybir.InstNoOp(
                            name=nc.get_next_instruction_name(),
                            sync_info=mybir.SyncInfo(on_wait=[w], on_update=[]),
                            bass_nofuse=True,
                            engine=inst.engine,
                        )
                        nc.register_instruction(nop, overwrite=True)
                        new.append(nop)
                    waits = waits[-1:]
                    inst.sync_info = mybir.SyncInfo(on_wait=waits, on_update=ups)
                    changed = True
                if si is not None and len(ups) > 1 and not is_dma:
                    for u in ups[1:]:
                        nop = mybir.InstNoOp(
                            name=nc.get_next_instruction_name(),
                            sync_info=mybir.SyncInfo(on_wait=[], on_update=[u]),
                            bass_nofuse=True,
                            engine=inst.engine,
                        )
                        nc.register_instruction(nop, overwrite=True)
                        post.append(nop)
                    inst.sync_info = mybir.SyncInfo(
                        on_wait=waits, on_update=ups[:1])
                    changed = True
                new.append(inst)
                new.extend(post)
            if changed:
                bb.instructions = new


def _u2bias():
    # U2[k, j] = EXP_BIAS if j >= k else MASK_BIAS, doubled side by side
    # so a [p, 2, span] strided AP serves both heads of a diagonal block.
    s = np.arange(P)[:, None]
    j = np.arange(QB)[None, :]
    u = np.where(j >= s, EXP_BIAS, MASK_BIAS).astype(np.float32)
    return np.concatenate([u, u], axis=1)


def _build_nc(with_bias, debug=False):
    import concourse.bass as bass
    import concourse.mybir as mybir
    from concourse.tile import TileContext

    f32 = mybir.dt.float32
    bf16 = mybir.dt.bfloat16
    i16 = mybir.dt.int16
    AF = mybir.ActivationFunctionType
    ALU = mybir.AluOpType

    nc = bass.Bass()

    xTp = nc.dram_tensor("xTp", [P, 4 * T], bf16, kind="ExternalInput")
    wvp = nc.dram_tensor("wvp", [P, C], bf16, kind="ExternalInput")
    wpT = nc.dram_tensor("wpT", [P, C], bf16, kind="ExternalInput")
    wrp = nc.dram_tensor("wrp", [P, T], bf16, kind="ExternalInput")
    smf = nc.dram_tensor("smf", [P, 1], f32, kind="ExternalInput")
    outT = nc.dram_tensor("outT", [C, T], bf16, kind="ExternalOutput")
    rscr = [nc.dram_tensor(f"rscr{i}", [2, QB], bf16, kind="Internal")
            for i in range(2)]
    if debug:
        dbg = {
            "dvT": nc.dram_tensor("dvT", [P, T], bf16,
                                  kind="ExternalOutput"),
            "dwvT": nc.dram_tensor("dwvT", [P, T], bf16,
                                   kind="ExternalOutput"),
            "dva": nc.dram_tensor("dva", [P, 4 * (2 * HD + 2)], bf16,
                                  kind="ExternalOutput"),
            "de": nc.dram_tensor("de", [P, 2 * QB], bf16,
                                 kind="ExternalOutput"),
            "de2": nc.dram_tensor("de2", [P, 2 * QB], bf16,
                                  kind="ExternalOutput"),
            "ddd": nc.dram_tensor("ddd", [33, QB], bf16,
                                  kind="ExternalOutput"),
            "dr1": nc.dram_tensor("dr1", [33, QB], bf16,
                                  kind="ExternalOutput"),
            "drr": nc.dram_tensor("drr", [P, QB], bf16,
                                  kind="ExternalOutput"),
            "dysb": nc.dram_tensor("dysb", [P, QB], bf16,
                                   kind="ExternalOutput"),
        }

    u2_d = nc.inline_tensor(_u2bias(), name="u2bias")
    mg_d = nc.inline_tensor(
        np.full((33, QB), MAGIC16, np.int16), name="magic16")

    arena = nc.alloc_psum_tensor("arena", [P, 4096], f32)

    # arena column layout (each 512-col slab = one PSUM bank)
    SCOL = (0, 1024)                 # S ping/pong, [128, 1024] each
    YCOL = (2048, 3072)              # yps pair per qj parity

    with TileContext(nc) as tc:
        with (
            tc.tile_pool(name="persist", bufs=1) as pp,
            tc.tile_pool(name="stream", bufs=2) as sp,
        ):
            # ---- persistent SBUF ----
            xT_sb = pp.tile([P, 4 * T], bf16, tag="xTp")
            vT_sb = pp.tile([P, T], bf16, tag="vT")
            wvT_sb = pp.tile([P, T], bf16, tag="wvT")
            wrp_sb = pp.tile([P, T], bf16, tag="wrp")
            wv_sb = pp.tile([P, C], bf16, tag="wvp")
            wpT_sb = pp.tile([P, C], bf16, tag="wp")
            va_sb = [[pp.tile([P, HD + 1], bf16, tag=f"va{i}_{u}",
                              name=f"va{i}_{u}") for u in (0, 1)]
                     for i in range(NKB)]
            u2_sb = pp.tile([P, 2 * QB], f32, tag="u2")
            mg_sb = pp.tile([33, QB], i16, tag="mg")
            smf_sb = pp.tile([P, 1], f32, tag="smf")
            bvc_sb = smf_sb[:, 0:1]

            # ---- prologue DMAs, split across sync + scalar queues ----
            xT3d = xTp.rearrange("p (k t) -> p k t", t=T)
            xT3s = xT_sb[:].rearrange("p (k t) -> p k t", t=T)
            nc.sync.dma_start(out=wv_sb[:], in_=wvp[:])
            for k in range(4):
                eng = nc.sync if k % 2 == 0 else nc.scalar
                eng.dma_start(out=xT3s[:, k, 0:QB], in_=xT3d[:, k, 0:QB])
            nc.scalar.dma_start(out=wrp_sb[:], in_=wrp[:])
            nc.sync.dma_start(out=u2_sb[:], in_=u2_d[:])
            for c in range(1, 4):
                eng = nc.sync if c % 2 == 1 else nc.scalar
                eng.dma_start(out=xT3s[:, :, c * QB:(c + 1) * QB],
                              in_=xT3d[:, :, c * QB:(c + 1) * QB])
            nc.scalar.dma_start(out=wpT_sb[:], in_=wpT[:])
            nc.sync.dma_start(out=mg_sb[:], in_=mg_d[:])
            if with_bias:
                nc.scalar.dma_start(out=smf_sb[:], in_=smf[:])
            for i in range(NKB):
                for u in (0, 1):
                    nc.gpsimd.memset(va_sb[i][u][:, HD:HD + 1], 1.0)

            # ---- PE warm-up: ~2.6us of dummy matmuls on wv while the
            # bulk DMAs land, so real matmuls start at the 2.4GHz clock.
            wps = arena[0:P, 0:P]
            for _ in range(22):
                nc.tensor.matmul(wps, wv_sb[:, 0:P], wv_sb[:, 0:P],
                                 start=True, stop=True)

            ebuf = {}
            ysb_t = {}
            r1_t = {}
            ot_t = {}

            def ycols(qj, u):
                base = YCOL[qj % 2]
                return slice(base + u * QB, base + (u + 1) * QB)

            def emit_VP(qj):
                vps = arena[0:P, YCOL[qj % 2]:YCOL[qj % 2] + QB]
                for k in range(4):
                    nc.tensor.matmul(
                        vps, wv_sb[:, k * P:(k + 1) * P],
                        xT_sb[:, k * T + qj * QB:k * T + (qj + 1) * QB],
                        start=(k == 0), stop=(k == 3))

            def emit_VC(qj):
                vps = arena[0:P, YCOL[qj % 2]:YCOL[qj % 2] + QB]
                dst = vT_sb[:, qj * QB:(qj + 1) * QB]
                if with_bias:
                    nc.scalar.activation(dst, vps, AF.Copy, bias=bvc_sb)
                else:
                    nc.scalar.copy(dst, vps)

            def emit_WV(qj):
                sl = slice(qj * QB, (qj + 1) * QB)
                nc.gpsimd.tensor_mul(wvT_sb[:, sl], vT_sb[:, sl],
                                     wrp_sb[:, sl])

            def emit_TR(qj):
                for g in range(4):
                    kb = 4 * qj + g
                    for u in (0, 1):
                        nc.sync.dma_start(
                            out=va_sb[kb][u][:, 0:HD],
                            in_=wvT_sb[HD * u:HD * (u + 1),
                                       kb * P:(kb + 1) * P],
                            transpose=True)

            def emit_QK(qj, ki, sbuf_i):
                diag = ki >= 4 * qj
                so = P * (ki - 4 * qj) if diag else 0
                c0 = SCOL[sbuf_i]
                spair = arena[0:P, c0:c0 + 2 * QB]
                for u in (0, 1):
                    nc.tensor.matmul(
                        spair[:, u * QB + so:(u + 1) * QB],
                        wvT_sb[HD * u:HD * (u + 1), ki * P:(ki + 1) * P],
                        vT_sb[HD * u:HD * (u + 1),
                              qj * QB + so:(qj + 1) * QB],
                        start=True, stop=True,
                        tile_position=(HD * u, 0))
                return spair, so

            def emit_EXP(qj, ki, spair, so, eng):
                diag = ki >= 4 * qj
                e = sp.tile([P, 2 * QB], bf16, tag="e", name=f"e{qj}_{ki}",
                            bufs=4)
                if eng == "act":
                    if not diag:
                        nc.scalar.activation(e[:], spair, AF.Exp,
                                             scale=0.125)
                    else:
                        e3 = e[:].rearrange("p (u q) -> p u q", q=QB)
                        s3 = spair.rearrange("p (u q) -> p u q", q=QB)
                        nc.scalar.activation(
                            e3[:, :, so:QB], s3[:, :, so:QB], AF.Exp,
                            scale=0.125)
                        u3 = u2_sb[:].rearrange("p (u q) -> p u q", q=QB)
                        # mask on gpsimd: e *= (bias>=MASK? ) -- use
                        # 0/1 trick: compare not avail; multiply by U
                        # is handled below via DVE fallback; ACT-diag
                        # not used by default policy.
                        raise AssertionError("ACT diag not supported")
                else:
                    ei = e[:].bitcast(i16)
                    if not diag:
                        nc.vector.tensor_scalar(
                            ei, spair, EXP_A / 8.0, EXP_BIAS,
                            ALU.mult, ALU.add)
                    else:
                        e3 = ei.rearrange("p (u q) -> p u q", q=QB)
                        s3 = spair.rearrange("p (u q) -> p u q", q=QB)
                        u3 = u2_sb[:].rearrange("p (u q) -> p u q", q=QB)
                        nc.vector.scalar_tensor_tensor(
                            e3[:, :, so:QB], s3[:, :, so:QB], EXP_A / 8.0,
                            u3[:, :, 0:QB - so], ALU.mult, ALU.add)
                if debug and (qj, ki) == (0, 0):
                    nc.sync.dma_start(out=dbg["de"][:], in_=e[:])
                if debug and (qj, ki) == (1, 0):
                    nc.sync.dma_start(out=dbg["de2"][:], in_=e[:])
                ebuf[(qj, ki)] = (e, so)

            def emit_AV(qj, ki):
                e, so = ebuf.pop((qj, ki))
                last = ki == 4 * qj + 3
                for u in (0, 1):
                    nc.tensor.matmul(
                        arena[0:HD + 1, ycols(qj, u)][:, so:QB],
                        va_sb[ki][u][:],
                        e[:, u * QB + so:(u + 1) * QB],
                        start=(ki == 0), stop=last)

            def emit_DCOPY(qj):
                # d rows land on partitions 0 and 32 (legal AP bases).
                dd = sp.tile([33, QB], bf16, tag="dd", name=f"dd{qj}",
                             bufs=2)
                nc.scalar.copy(dd[0:1, :], arena[HD:HD + 1, ycols(qj, 0)])
                nc.vector.tensor_copy(dd[32:33, :],
                                      arena[HD:HD + 1, ycols(qj, 1)])
                if debug and qj == 0:
                    nc.sync.dma_start(out=dbg["ddd"][:], in_=dd[:])
                return dd

            def emit_CHAIN(qj, dd):
                # -1/d in bf16: magic seed + one Newton step.
                # r0 = bitcast(magic - bits(d)); t = d*r0; r1n = (t-2)*r0
                r0 = sp.tile([33, QB], bf16, tag="r0", name=f"r0_{qj}",
                             bufs=2)
                nc.vector.tensor_tensor(r0[:].bitcast(i16), mg_sb[:],
                                        dd[:].bitcast(i16), ALU.subtract)
                t = sp.tile([33, QB], bf16, tag="rt", name=f"rt{qj}",
                            bufs=2)
                nc.vector.tensor_mul(t[:], dd[:], r0[:])
                r1n = sp.tile([33, QB], bf16, tag="r1", name=f"r1_{qj}",
                              bufs=2)
                nc.vector.scalar_tensor_tensor(
                    r1n[:], t[:], 2.0, r0[:], ALU.subtract, ALU.mult)
                if debug and qj == 0:
                    nc.sync.dma_start(out=dbg["dr1"][:], in_=r1n[:])
                r1_t[qj] = r1n

            def emit_RREP(qj):
                # partition-broadcast via a DRAM bounce: stride-0 SBUF
                # partition APs are rejected, DRAM-side broadcast works.
                r1n = r1_t.pop(qj)
                rrep = sp.tile([P, QB], bf16, tag="rr", name=f"rr{qj}",
                               bufs=2)
                scr = rscr[qj % 2]
                nc.sync.dma_start(out=scr[0:1, :], in_=r1n[0:1, :])
                nc.scalar.dma_start(out=scr[1:2, :], in_=r1n[32:33, :])
                nc.sync.dma_start(
                    out=rrep[0:HD, :],
                    in_=scr[0:1, :].broadcast_to([HD, QB]))
                nc.scalar.dma_start(
                    out=rrep[HD:P, :],
                    in_=scr[1:2, :].broadcast_to([HD, QB]))
                if debug and qj == 0:
                    nc.sync.dma_start(out=dbg["drr"][:], in_=rrep[:])
                return rrep

            def emit_YRAW(qj):
                # evacuate unnormalized y early: frees the yps psum pair
                # without waiting for the reciprocal round-trip.
                yraw = sp.tile([P, QB], bf16, tag="yw", name=f"yw{qj}",
                               bufs=2)
                nc.scalar.copy(yraw[0:HD, :], arena[0:HD, ycols(qj, 0)])
                nc.vector.tensor_copy(yraw[HD:P, :],
                                      arena[0:HD, ycols(qj, 1)])
                return yraw

            def emit_YMUL(qj, yraw, rrep):
                ysb = sp.tile([P, QB], bf16, tag="y", name=f"ysb{qj}",
                              bufs=2)
                nc.vector.tensor_mul(ysb[:], yraw[:], rrep[:])
                if debug and qj == 0:
                    nc.sync.dma_start(out=dbg["dysb"][:], in_=ysb[:])
                ysb_t[qj] = ysb

            def emit_OP(qj, j):
                # out-proj c-chunk j for query chunk qj; psum reuses the
                # (qj)%2... note: emitted during chunk qj+1, whose parity
                # pair (qj%2) was freed by YN(qj).
                base = YCOL[qj % 2] + (j % 2) * QB
                ops = arena[0:P, base:base + QB]
                nc.tensor.matmul(ops, wpT_sb[:, j * P:(j + 1) * P],
                                 ysb_t[qj][:], start=True, stop=True)

            def emit_OT(qj, pair):
                # evacuate op psum pair (2j, 2j+1) as one [128,1024] copy
                base = YCOL[qj % 2]
                src = arena[0:P, base:base + 2 * QB]
                ot = sp.tile([P, 2 * QB], bf16, tag="ot",
                             name=f"ot{qj}_{pair}", bufs=2)
                if pair == 0:
                    nc.scalar.copy(ot[:], src)
                else:
                    nc.vector.tensor_copy(ot[:], src)
                ot_t[(qj, pair)] = ot

            outT3 = outT.rearrange("(k p) t -> p k t", p=P)

            def emit_OD(qj, pair):
                ot = ot_t.pop((qj, pair))
                nc.sync.dma_start(
                    out=outT3[:, 2 * pair:2 * pair + 2,
                              qj * QB:(qj + 1) * QB],
                    in_=ot[:].rearrange("p (k t) -> p k t", t=QB))

            # exp engine policy: diagonal -> DVE (fused mask); off-diag
            # mostly ACT, every 8th to DVE for balance.
            od_counter = [0]

            def exp_engine(qj, ki):
                if ki >= 4 * qj:
                    return "dve"
                od_counter[0] += 1
                return "dve" if od_counter[0] % 8 == 0 else "act"

            # ---- software-pipelined schedule ----
            emit_VP(0)
            emit_VC(0)
            emit_WV(0)
            emit_TR(0)

            def boundary_extras(pq, qj):
                """Ordered (slot, thunk) list: recip/norm/out-proj for
                chunk pq, and V-path prefetch for chunk qj+1."""
                st = {}
                ex = [
                    (1, lambda: st.__setitem__("dd", emit_DCOPY(pq))),
                    (2, lambda: emit_CHAIN(pq, st["dd"])),
                    (3, lambda: (st.__setitem__("rr", emit_RREP(pq)),
                                 st.__setitem__("yw", emit_YRAW(pq)))),
                ]
                if qj <= 2:
                    ex += [
                        (4, lambda: emit_VP(qj + 1)),
                        (5, lambda: (emit_VC(qj + 1), emit_WV(qj + 1))),
                        (6, lambda: emit_TR(qj + 1)),
                    ]
                # the normalize multiply and out-proj sit late so the
                # in-order PE/DVE queues never park behind the rrep DMA
                # round-trip.
                ex += [
                    (8, lambda: emit_YMUL(pq, st["yw"], st["rr"])),
                    (9, lambda: emit_OP(pq, 0)),
                    (10, lambda: emit_OP(pq, 1)),
                    (11, lambda: (emit_OT(pq, 0), emit_OP(pq, 2))),
                    (12, lambda: (emit_OP(pq, 3), emit_OD(pq, 0))),
                    (13, lambda: emit_OT(pq, 1)),
                    (14, lambda: (emit_OD(pq, 1), ysb_t.pop(pq))),
                ]
                return ex

            for qj in range(NQ):
                nki = 4 * qj + 4
                if qj == 0:
                    extras = [(1, lambda: emit_VP(1)),
                              (2, lambda: (emit_VC(1), emit_WV(1))),
                              (3, lambda: emit_TR(1))]
                else:
                    extras = boundary_extras(qj - 1, qj)
                for i in range(nki + 1):
                    if i < nki:
                        spair, so = emit_QK(qj, i, i % 2)
                        emit_EXP(qj, i, spair, so, exp_engine(qj, i))
                    if 1 <= i <= nki:
                        emit_AV(qj, i - 1)
                    while extras and extras[0][0] <= i:
                        extras.pop(0)[1]()
                for _, thunk in extras:
                    thunk()

            # ---- epilogue for the last chunk ----
            for _, thunk in boundary_extras(NQ - 1, NQ - 1):
                thunk()
            if debug:
                nc.sync.dma_start(out=dbg["dvT"][:], in_=vT_sb[:])
                nc.sync.dma_start(out=dbg["dwvT"][:], in_=wvT_sb[:])
                dva3 = dbg["dva"].rearrange("p (k u c) -> p k u c",
                                            c=HD + 1, u=2)
                for kb in range(4):
                    for u in (0, 1):
                        nc.sync.dma_start(out=dva3[:, kb, u, :],
                                          in_=va_sb[kb][u][:])

    import concourse.mybir as mybir2
    _split_multi_waits(nc, mybir2)
    return nc


def _get_nc(with_bias=False, debug=False):
    key = f"nc{int(with_bias)}{int(debug)}"
    if key not in _cache:
        _cache[key] = _build_nc(with_bias, debug)
    return _cache[key]


def _make_in_maps(x, weight, Wv, bv, Wp, bp, state):
    x = np.asarray(x, np.float32)
    w = np.asarray(weight, np.float32)[:, :, 0]
    if not int(np.asarray(state)):
        w = np.ones_like(w)
    WvT = np.asarray(Wv, np.float32).T
    WpTn = -np.asarray(Wp, np.float32).T      # negated: folds -1/d sign
    bv = np.asarray(bv, np.float32)

    in_maps = []
    for core in range(8):
        b, hp = core // 4, core % 4
        js = slice(P * hp, P * (hp + 1))
        xTb = x[b].T.reshape(4, P, T).transpose(1, 0, 2).reshape(P, 4 * T)
        wvpb = WvT[:, js].reshape(4, P, P).transpose(1, 0, 2).reshape(P, C)
        wrpb = np.broadcast_to(w[b][None, :], (P, T))
        smfb = bv[js].reshape(P, 1)
        in_maps.append({
            "xTp": np.ascontiguousarray(xTb).astype(ml_dtypes.bfloat16),
            "wvp": np.ascontiguousarray(wvpb).astype(ml_dtypes.bfloat16),
            "wpT": np.ascontiguousarray(WpTn[js, :]).astype(
                ml_dtypes.bfloat16),
            "wrp": np.ascontiguousarray(wrpb).astype(ml_dtypes.bfloat16),
            "smf": np.ascontiguousarray(smfb),
        })
    return in_maps


def _gather(results, x=None, bp=None):
    out = np.empty((B, T, C), np.float32)
    for b in range(B):
        acc = np.zeros((C, T), np.float32)
        for hp in range(4):
            acc += results[4 * b + hp]["outT"].astype(np.float32)
        out[b] = acc.T
    if bp is not None:
        out += np.asarray(bp, np.float32)[None, None, :]
    return out


def _run(in_maps, with_bias=False, debug=False, **kw):
    from concourse.bass_utils import run_bass_kernel_spmd
    return run_bass_kernel_spmd(
        _get_nc(with_bias, debug), in_maps, list(range(8)), **kw)


def kernel(x, weight, Wv, bv, Wp, bp, state):
    in_maps = _make_in_maps(x, weight, Wv, bv, Wp, bp, state)
    res = _run(in_maps, with_bias=bool(np.any(np.asarray(bv))))
    return _gather(res.results, x, bp)


# revision 18
# speedup vs baseline: 1.2087x; 1.2087x over previous
"""Causal self-attention (weight-modulated) Trainium2 kernel, 8-core SPMD.

Reference semantics (B=2, T=2048, C=512, 8 heads, hd=64):
    v0  = x @ Wv.T + bv
    att = softmax(mask((v0h @ v0h^T) * w[key] / sqrt(hd)))
    y   = att @ (v0*w[row])h
    out = y @ Wp.T + bp

Sharding: core = (b, hp) with b = batch, hp = head pair (v0 dims
[128hp, 128hp+128)).  v2 design notes:
  - w[key] is folded into wvT = vT * wrep (GpSimd), so QK against wvT
    yields pre-scaled scores and va = DMA-transpose(wvT) needs no
    further scaling.
  - exp is split across ACT (exact, off-diag blocks) and DVE
    (Schraudolph bf16 bit-trick: i16 = S*A + bias, bitcast; the causal
    mask of diagonal blocks is fused via scalar_tensor_tensor with a
    bias/mask tile: masked lanes get +4000 -> bitcast ~1e-25 ~ 0).
  - QK row-tiled: the two heads' K=64 matmuls run concurrently on
    array row strips (tile_position (0,0)/(64,0)).
  - softmax denominator rides as a 65th ones-column in the AV
    stationary; 1/d via bf16 magic-seed + 1 Newton step on DVE; the
    (negated) reciprocal is partition-broadcast by GpSimd and the sign
    is folded into a host-negated Wp.
  - PSUM is hand-placed in one [128, 4096] arena: banks 0-3 = S
    ping/pong, banks 4-7 = two yps pairs (alternating qj parity), with
    V-proj and out-proj matmuls reusing freed yps banks.
Host: out[b] = sum_hp outT^T + bp  (partial-sum reduce off-device).
"""

import ml_dtypes
import numpy as np

B, T, C = 2, 2048, 512
NH, HD = 8, 64
P = 128
QB = 512                 # query chunk
NQ = 4                   # query chunks
NKB = 16                 # key blocks of 128

EXP_A = 128 * 1.4426950408889634     # ln->bf16-exponent scale
EXP_BIAS = 16250.625                 # 16256 - C (C calibrated)
MASK_BIAS = 4000.0                   # masked lanes -> tiny positive
MAGIC16 = 0x7EF3                     # bf16 reciprocal seed

_cache = {}


def _split_multi_waits(nc, mybir):
    """Walrus in this container encodes at most ONE sync wait (and one
    update) per instruction; Tile's sem assignment emits several. Hoist
    excess waits onto single-wait NOPs placed just before the
    instruction on the same engine, and excess updates of non-DMA
    instructions onto NOPs just after."""
    dma_ops = {"DMACopy", "DMATranspose", "TensorCopy"}
    for f in nc.m.functions:
        for bb in f.blocks:
            new = []
            changed = False
            for inst in bb.instructions:
                si = inst.sync_info
                waits = list(si.on_wait or []) if si is not None else []
                ups = list(si.on_update or []) if si is not None else []
                is_dma = inst.concise_opcode() in dma_ops if hasattr(
                    inst, "concise_opcode") else False
                post = []
                if si is not None and len(waits) > 1:
                    for w in waits[:-1]:
                        nop = mybir.InstNoOp(
                            name=nc.get_next_instruction_name(),
                            sync_info=mybir.SyncInfo(on_wait=[w], on_update=[]),
                            bass_nofuse=True,
                            engine=inst.engine,
                        )
                        nc.register_instruction(nop, overwrite=True)
                        new.append(nop)
                    waits = waits[-1:]
                    inst.sync_info = mybir.SyncInfo(on_wait=waits, on_update=ups)
                    changed = True
                if si is not None and len(ups) > 1 and not is_dma:
                    for u in ups[1:]:
                        nop = mybir.InstNoOp(
                            name=nc.get_next_instruction_name(),
                            sync_info=mybir.SyncInfo(on_wait=[], on_update=[u]),
                            bass_nofuse=True,
                            engine=inst.engine,
                        )
                        nc.register_instruction(nop, overwrite=True)
                        post.append(nop)
                    inst.sync_info = mybir.SyncInfo(
                        on_wait=waits, on_update=ups[:1])
                    changed = True
                new.append(inst)
                new.extend(post)
            if changed:
                bb.instructions = new


def _u2bias():
    # U2[k, j] = EXP_BIAS if j >= k else MASK_BIAS, doubled side by side
    # so a [p, 2, span] strided AP serves both heads of a diagonal block.
    s = np.arange(P)[:, None]
    j = np.arange(QB)[None, :]
    u = np.where(j >= s, EXP_BIAS, MASK_BIAS).astype(np.float32)
    return np.concatenate([u, u], axis=1)


def _build_nc(with_bias, debug=False):
    import concourse.bass as bass
    import concourse.mybir as mybir
    from concourse.tile import TileContext

    f32 = mybir.dt.float32
    bf16 = mybir.dt.bfloat16
    i16 = mybir.dt.int16
    AF = mybir.ActivationFunctionType
    ALU = mybir.AluOpType

    nc = bass.Bass()

    xTp = nc.dram_tensor("xTp", [P, 4 * T], bf16, kind="ExternalInput")
    wvp = nc.dram_tensor("wvp", [P, C], bf16, kind="ExternalInput")
    wpT = nc.dram_tensor("wpT", [P, C], bf16, kind="ExternalInput")
    wrp = nc.dram_tensor("wrp", [P, T], bf16, kind="ExternalInput")
    smf = nc.dram_tensor("smf", [P, 1], f32, kind="ExternalInput")
    outT = nc.dram_tensor("outT", [C, T], bf16, kind="ExternalOutput")
    rscr = [nc.dram_tensor(f"rscr{i}", [2, QB], bf16, kind="Internal")
            for i in range(2)]
    if debug:
        dbg = {
            "dvT": nc.dram_tensor("dvT", [P, T], bf16,
                                  kind="ExternalOutput"),
            "dwvT": nc.dram_tensor("dwvT", [P, T], bf16,
                                   kind="ExternalOutput"),
            "dva": nc.dram_tensor("dva", [P, 4 * (2 * HD + 2)], bf16,
                                  kind="ExternalOutput"),
            "de": nc.dram_tensor("de", [P, 2 * QB], bf16,
                                 kind="ExternalOutput"),
            "de2": nc.dram_tensor("de2", [P, 2 * QB], bf16,
                                  kind="ExternalOutput"),
            "ddd": nc.dram_tensor("ddd", [33, QB], bf16,
                                  kind="ExternalOutput"),
            "dr1": nc.dram_tensor("dr1", [33, QB], bf16,
                                  kind="ExternalOutput"),
            "drr": nc.dram_tensor("drr", [P, QB], bf16,
                                  kind="ExternalOutput"),
            "dysb": nc.dram_tensor("dysb", [P, QB], bf16,
                                   kind="ExternalOutput"),
        }

    u2_d = nc.inline_tensor(_u2bias(), name="u2bias")
    id_d = nc.inline_tensor(np.eye(P).astype(ml_dtypes.bfloat16),
                            name="idn")
    mg_d = nc.inline_tensor(
        np.full((33, QB), MAGIC16, np.int16), name="magic16")

    arena = nc.alloc_psum_tensor("arena", [P, 4096], f32)

    # arena column layout (each 512-col slab = one PSUM bank)
    SCOL = (0, 1024)                 # S ping/pong, [128, 1024] each
    YCOL = (2048, 3072)              # yps pair per qj parity

    with TileContext(nc) as tc:
        with (
            tc.tile_pool(name="persist", bufs=1) as pp,
            tc.tile_pool(name="stream", bufs=2) as sp,
        ):
            # ---- persistent SBUF ----
            xT_sb = pp.tile([P, 4 * T], bf16, tag="xTp")
            vT_sb = pp.tile([P, T], bf16, tag="vT")
            wvT_sb = pp.tile([P, T], bf16, tag="wvT")
            wrp_sb = pp.tile([P, T], bf16, tag="wrp")
            wv_sb = pp.tile([P, C], bf16, tag="wvp")
            wpT_sb = pp.tile([P, C], bf16, tag="wp")
            va_sb = [[pp.tile([P, HD + 1], bf16, tag=f"va{i}_{u}",
                              name=f"va{i}_{u}") for u in (0, 1)]
                     for i in range(NKB)]
            u2_sb = pp.tile([P, 2 * QB], f32, tag="u2")
            idn_sb = pp.tile([P, P], bf16, tag="idn")
            mg_sb = pp.tile([33, QB], i16, tag="mg")
            smf_sb = pp.tile([P, 1], f32, tag="smf")
            bvc_sb = smf_sb[:, 0:1]

            # ---- prologue DMAs, split across sync + scalar queues ----
            xT3d = xTp.rearrange("p (k t) -> p k t", t=T)
            xT3s = xT_sb[:].rearrange("p (k t) -> p k t", t=T)
            nc.sync.dma_start(out=wv_sb[:], in_=wvp[:])
            for k in range(4):
                eng = nc.sync if k % 2 == 0 else nc.scalar
                eng.dma_start(out=xT3s[:, k, 0:QB], in_=xT3d[:, k, 0:QB])
            nc.scalar.dma_start(out=wrp_sb[:], in_=wrp[:])
            nc.sync.dma_start(out=u2_sb[:], in_=u2_d[:])
            nc.scalar.dma_start(out=idn_sb[:], in_=id_d[:])
            for c in range(1, 4):
                eng = nc.sync if c % 2 == 1 else nc.scalar
                eng.dma_start(out=xT3s[:, :, c * QB:(c + 1) * QB],
                              in_=xT3d[:, :, c * QB:(c + 1) * QB])
            nc.scalar.dma_start(out=wpT_sb[:], in_=wpT[:])
            nc.sync.dma_start(out=mg_sb[:], in_=mg_d[:])
            if with_bias:
                nc.scalar.dma_start(out=smf_sb[:], in_=smf[:])
            for i in range(NKB):
                for u in (0, 1):
                    nc.gpsimd.memset(va_sb[i][u][:, HD:HD + 1], 1.0)

            # ---- PE warm-up: ~2.6us of dummy matmuls on wv while the
            # bulk DMAs land, so real matmuls start at the 2.4GHz clock.
            wps = arena[0:P, 0:P]
            for _ in range(22):
                nc.tensor.matmul(wps, wv_sb[:, 0:P], wv_sb[:, 0:P],
                                 start=True, stop=True)

            ebuf = {}
            ysb_t = {}
            r1_t = {}
            ot_t = {}

            def ycols(qj, u):
                base = YCOL[qj % 2]
                return slice(base + u * QB, base + (u + 1) * QB)

            def emit_VP(qj):
                vps = arena[0:P, YCOL[qj % 2]:YCOL[qj % 2] + QB]
                for k in range(4):
                    nc.tensor.matmul(
                        vps, wv_sb[:, k * P:(k + 1) * P],
                        xT_sb[:, k * T + qj * QB:k * T + (qj + 1) * QB],
                        start=(k == 0), stop=(k == 3))

            def emit_VC(qj):
                vps = arena[0:P, YCOL[qj % 2]:YCOL[qj % 2] + QB]
                dst = vT_sb[:, qj * QB:(qj + 1) * QB]
                if with_bias:
                    nc.scalar.activation(dst, vps, AF.Copy, bias=bvc_sb)
                else:
                    nc.scalar.copy(dst, vps)

            def emit_WV(qj):
                sl = slice(qj * QB, (qj + 1) * QB)
                nc.gpsimd.tensor_mul(wvT_sb[:, sl], vT_sb[:, sl],
                                     wrp_sb[:, sl])

            def emit_TR(qj):
                # PE-transpose each key block of wvT into transient bf16
                # psum (free half of chunk-qj's yps pair), then evacuate
                # per-head halves into the va tiles (ACT u0 / DVE u1).
                tbase = YCOL[qj % 2] + QB
                for g in range(4):
                    kb = 4 * qj + g
                    tps = arena[0:P, tbase + g * HD:
                                tbase + (g + 1) * HD].bitcast(bf16)
                    nc.tensor.transpose(
                        tps, wvT_sb[:, kb * P:(kb + 1) * P], idn_sb[:])
                    nc.scalar.copy(va_sb[kb][0][:, 0:HD], tps[:, 0:HD])
                    nc.vector.tensor_copy(va_sb[kb][1][:, 0:HD],
                                          tps[:, HD:P])

            def emit_QK(qj, ki, sbuf_i):
                diag = ki >= 4 * qj
                so = P * (ki - 4 * qj) if diag else 0
                c0 = SCOL[sbuf_i]
                spair = arena[0:P, c0:c0 + 2 * QB]
                for u in (0, 1):
                    nc.tensor.matmul(
                        spair[:, u * QB + so:(u + 1) * QB],
                        wvT_sb[HD * u:HD * (u + 1), ki * P:(ki + 1) * P],
                        vT_sb[HD * u:HD * (u + 1),
                              qj * QB + so:(qj + 1) * QB],
                        start=True, stop=True,
                        tile_position=(HD * u, 0))
                return spair, so

            def emit_EXP(qj, ki, spair, so, eng):
                diag = ki >= 4 * qj
                e = sp.tile([P, 2 * QB], bf16, tag="e", name=f"e{qj}_{ki}",
                            bufs=4)
                if eng == "act":
                    if not diag:
                        nc.scalar.activation(e[:], spair, AF.Exp,
                                             scale=0.125)
                    else:
                        e3 = e[:].rearrange("p (u q) -> p u q", q=QB)
                        s3 = spair.rearrange("p (u q) -> p u q", q=QB)
                        nc.scalar.activation(
                            e3[:, :, so:QB], s3[:, :, so:QB], AF.Exp,
                            scale=0.125)
                        u3 = u2_sb[:].rearrange("p (u q) -> p u q", q=QB)
                        # mask on gpsimd: e *= (bias>=MASK? ) -- use
                        # 0/1 trick: compare not avail; multiply by U
                        # is handled below via DVE fallback; ACT-diag
                        # not used by default policy.
                        raise AssertionError("ACT diag not supported")
                else:
                    ei = e[:].bitcast(i16)
                    if not diag:
                        nc.vector.tensor_scalar(
                            ei, spair, EXP_A / 8.0, EXP_BIAS,
                            ALU.mult, ALU.add)
                    else:
                        e3 = ei.rearrange("p (u q) -> p u q", q=QB)
                        s3 = spair.rearrange("p (u q) -> p u q", q=QB)
                        u3 = u2_sb[:].rearrange("p (u q) -> p u q", q=QB)
                        nc.vector.scalar_tensor_tensor(
                            e3[:, :, so:QB], s3[:, :, so:QB], EXP_A / 8.0,
                            u3[:, :, 0:QB - so], ALU.mult, ALU.add)
                if debug and (qj, ki) == (0, 0):
                    nc.sync.dma_start(out=dbg["de"][:], in_=e[:])
                if debug and (qj, ki) == (1, 0):
                    nc.sync.dma_start(out=dbg["de2"][:], in_=e[:])
                ebuf[(qj, ki)] = (e, so)

            def emit_AV(qj, ki):
                e, so = ebuf.pop((qj, ki))
                last = ki == 4 * qj + 3
                for u in (0, 1):
                    nc.tensor.matmul(
                        arena[0:HD + 1, ycols(qj, u)][:, so:QB],
                        va_sb[ki][u][:],
                        e[:, u * QB + so:(u + 1) * QB],
                        start=(ki == 0), stop=last)

            def emit_DCOPY(qj):
                # d rows land on partitions 0 and 32 (legal AP bases).
                dd = sp.tile([33, QB], bf16, tag="dd", name=f"dd{qj}",
                             bufs=2)
                nc.scalar.copy(dd[0:1, :], arena[HD:HD + 1, ycols(qj, 0)])
                nc.vector.tensor_copy(dd[32:33, :],
                                      arena[HD:HD + 1, ycols(qj, 1)])
                if debug and qj == 0:
                    nc.sync.dma_start(out=dbg["ddd"][:], in_=dd[:])
                return dd

            def emit_CHAIN(qj, dd):
                # -1/d in bf16: magic seed + one Newton step.
                # r0 = bitcast(magic - bits(d)); t = d*r0; r1n = (t-2)*r0
                r0 = sp.tile([33, QB], bf16, tag="r0", name=f"r0_{qj}",
                             bufs=2)
                nc.vector.tensor_tensor(r0[:].bitcast(i16), mg_sb[:],
                                        dd[:].bitcast(i16), ALU.subtract)
                t = sp.tile([33, QB], bf16, tag="rt", name=f"rt{qj}",
                            bufs=2)
                nc.vector.tensor_mul(t[:], dd[:], r0[:])
                r1n = sp.tile([33, QB], bf16, tag="r1", name=f"r1_{qj}",
                              bufs=2)
                nc.vector.scalar_tensor_tensor(
                    r1n[:], t[:], 2.0, r0[:], ALU.subtract, ALU.mult)
                if debug and qj == 0:
                    nc.sync.dma_start(out=dbg["dr1"][:], in_=r1n[:])
                r1_t[qj] = r1n

            def emit_RREP(qj):
                # partition-broadcast via a DRAM bounce: stride-0 SBUF
                # partition APs are rejected, DRAM-side broadcast works.
                r1n = r1_t.pop(qj)
                rrep = sp.tile([P, QB], bf16, tag="rr", name=f"rr{qj}",
                               bufs=2)
                scr = rscr[qj % 2]
                nc.sync.dma_start(out=scr[0:1, :], in_=r1n[0:1, :])
                nc.scalar.dma_start(out=scr[1:2, :], in_=r1n[32:33, :])
                nc.sync.dma_start(
                    out=rrep[0:HD, :],
                    in_=scr[0:1, :].broadcast_to([HD, QB]))
                nc.scalar.dma_start(
                    out=rrep[HD:P, :],
                    in_=scr[1:2, :].broadcast_to([HD, QB]))
                if debug and qj == 0:
                    nc.sync.dma_start(out=dbg["drr"][:], in_=rrep[:])
                return rrep

            def emit_YRAW(qj):
                # evacuate unnormalized y early: frees the yps psum pair
                # without waiting for the reciprocal round-trip.
                yraw = sp.tile([P, QB], bf16, tag="yw", name=f"yw{qj}",
                               bufs=2)
                nc.scalar.copy(yraw[0:HD, :], arena[0:HD, ycols(qj, 0)])
                nc.vector.tensor_copy(yraw[HD:P, :],
                                      arena[0:HD, ycols(qj, 1)])
                return yraw

            def emit_YMUL(qj, yraw, rrep):
                ysb = sp.tile([P, QB], bf16, tag="y", name=f"ysb{qj}",
                              bufs=2)
                nc.vector.tensor_mul(ysb[:], yraw[:], rrep[:])
                if debug and qj == 0:
                    nc.sync.dma_start(out=dbg["dysb"][:], in_=ysb[:])
                ysb_t[qj] = ysb

            def emit_OP(qj, j):
                # out-proj c-chunk j for query chunk qj; psum reuses the
                # (qj)%2... note: emitted during chunk qj+1, whose parity
                # pair (qj%2) was freed by YN(qj).
                base = YCOL[qj % 2] + (j % 2) * QB
                ops = arena[0:P, base:base + QB]
                nc.tensor.matmul(ops, wpT_sb[:, j * P:(j + 1) * P],
                                 ysb_t[qj][:], start=True, stop=True)

            def emit_OT(qj, pair):
                # evacuate op psum pair (2j, 2j+1) as one [128,1024] copy
                base = YCOL[qj % 2]
                src = arena[0:P, base:base + 2 * QB]
                ot = sp.tile([P, 2 * QB], bf16, tag="ot",
                             name=f"ot{qj}_{pair}", bufs=2)
                if pair == 0:
                    nc.scalar.copy(ot[:], src)
                else:
                    nc.vector.tensor_copy(ot[:], src)
                ot_t[(qj, pair)] = ot

            outT3 = outT.rearrange("(k p) t -> p k t", p=P)

            def emit_OD(qj, pair):
                ot = ot_t.pop((qj, pair))
                nc.sync.dma_start(
                    out=outT3[:, 2 * pair:2 * pair + 2,
                              qj * QB:(qj + 1) * QB],
                    in_=ot[:].rearrange("p (k t) -> p k t", t=QB))

            # exp engine policy: diagonal -> DVE (fused mask); off-diag
            # mostly ACT, every 8th to DVE for balance.
            od_counter = [0]

            def exp_engine(qj, ki):
                if ki >= 4 * qj:
                    return "dve"
                od_counter[0] += 1
                return "dve" if od_counter[0] % 8 == 0 else "act"

            # ---- software-pipelined schedule ----
            emit_VP(0)
            emit_VC(0)
            emit_WV(0)
            emit_TR(0)

            def boundary_extras(pq, qj):
                """Ordered (slot, thunk) list: recip/norm/out-proj for
                chunk pq, and V-path prefetch for chunk qj+1."""
                st = {}
                ex = [
                    (1, lambda: st.__setitem__("dd", emit_DCOPY(pq))),
                    (2, lambda: emit_CHAIN(pq, st["dd"])),
                    (3, lambda: (st.__setitem__("rr", emit_RREP(pq)),
                                 st.__setitem__("yw", emit_YRAW(pq)))),
                ]
                if qj <= 2:
                    ex += [
                        (4, lambda: emit_VP(qj + 1)),
                        (5, lambda: (emit_VC(qj + 1), emit_WV(qj + 1))),
                        (6, lambda: emit_TR(qj + 1)),
                    ]
                # the normalize multiply and out-proj sit late so the
                # in-order PE/DVE queues never park behind the rrep DMA
                # round-trip.
                ex += [
                    (8, lambda: emit_YMUL(pq, st["yw"], st["rr"])),
                    (9, lambda: emit_OP(pq, 0)),
                    (10, lambda: emit_OP(pq, 1)),
                    (11, lambda: (emit_OT(pq, 0), emit_OP(pq, 2))),
                    (12, lambda: (emit_OP(pq, 3), emit_OD(pq, 0))),
                    (13, lambda: emit_OT(pq, 1)),
                    (14, lambda: (emit_OD(pq, 1), ysb_t.pop(pq))),
                ]
                return ex

            for qj in range(NQ):
                nki = 4 * qj + 4
                if qj == 0:
                    extras = [(1, lambda: emit_VP(1)),
                              (2, lambda: (emit_VC(1), emit_WV(1))),
                              (3, lambda: emit_TR(1))]
                else:
                    extras = boundary_extras(qj - 1, qj)
                for i in range(nki + 1):
                    if i < nki:
                        spair, so = emit_QK(qj, i, i % 2)
                        emit_EXP(qj, i, spair, so, exp_engine(qj, i))
                    if 1 <= i <= nki:
                        emit_AV(qj, i - 1)
                    while extras and extras[0][0] <= i:
                        extras.pop(0)[1]()
                for _, thunk in extras:
                    thunk()

            # ---- epilogue for the last chunk ----
            for _, thunk in boundary_extras(NQ - 1, NQ - 1):
                thunk()
            if debug:
                nc.sync.dma_start(out=dbg["dvT"][:], in_=vT_sb[:])
                nc.sync.dma_start(out=dbg["dwvT"][:], in_=wvT_sb[:])
                dva3 = dbg["dva"].rearrange("p (k u c) -> p k u c",
                                            c=HD + 1, u=2)
                for kb in range(4):
                    for u in (0, 1):
                        nc.sync.dma_start(out=dva3[:, kb, u, :],
                                          in_=va_sb[kb][u][:])

    import concourse.mybir as mybir2
    _split_multi_waits(nc, mybir2)
    return nc


def _get_nc(with_bias=False, debug=False):
    key = f"nc{int(with_bias)}{int(debug)}"
    if key not in _cache:
        _cache[key] = _build_nc(with_bias, debug)
    return _cache[key]


def _make_in_maps(x, weight, Wv, bv, Wp, bp, state):
    x = np.asarray(x, np.float32)
    w = np.asarray(weight, np.float32)[:, :, 0]
    if not int(np.asarray(state)):
        w = np.ones_like(w)
    WvT = np.asarray(Wv, np.float32).T
    WpTn = -np.asarray(Wp, np.float32).T      # negated: folds -1/d sign
    bv = np.asarray(bv, np.float32)

    in_maps = []
    for core in range(8):
        b, hp = core // 4, core % 4
        js = slice(P * hp, P * (hp + 1))
        xTb = x[b].T.reshape(4, P, T).transpose(1, 0, 2).reshape(P, 4 * T)
        wvpb = WvT[:, js].reshape(4, P, P).transpose(1, 0, 2).reshape(P, C)
        wrpb = np.broadcast_to(w[b][None, :], (P, T))
        smfb = bv[js].reshape(P, 1)
        in_maps.append({
            "xTp": np.ascontiguousarray(xTb).astype(ml_dtypes.bfloat16),
            "wvp": np.ascontiguousarray(wvpb).astype(ml_dtypes.bfloat16),
            "wpT": np.ascontiguousarray(WpTn[js, :]).astype(
                ml_dtypes.bfloat16),
            "wrp": np.ascontiguousarray(wrpb).astype(ml_dtypes.bfloat16),
            "smf": np.ascontiguousarray(smfb),
        })
    return in_maps


def _gather(results, x=None, bp=None):
    out = np.empty((B, T, C), np.float32)
    for b in range(B):
        acc = np.zeros((C, T), np.float32)
        for hp in range(4):
            acc += results[4 * b + hp]["outT"].astype(np.float32)
        out[b] = acc.T
    if bp is not None:
        out += np.asarray(bp, np.float32)[None, None, :]
    return out


def _run(in_maps, with_bias=False, debug=False, **kw):
    from concourse.bass_utils import run_bass_kernel_spmd
    return run_bass_kernel_spmd(
        _get_nc(with_bias, debug), in_maps, list(range(8)), **kw)


def kernel(x, weight, Wv, bv, Wp, bp, state):
    in_maps = _make_in_maps(x, weight, Wv, bv, Wp, bp, state)
    res = _run(in_maps, with_bias=bool(np.any(np.asarray(bv))))
    return _gather(res.results, x, bp)


# revision 19
# speedup vs baseline: 1.2354x; 1.0221x over previous
"""Causal self-attention (weight-modulated) Trainium2 kernel, 8-core SPMD.

Reference semantics (B=2, T=2048, C=512, 8 heads, hd=64):
    v0  = x @ Wv.T + bv
    att = softmax(mask((v0h @ v0h^T) * w[key] / sqrt(hd)))
    y   = att @ (v0*w[row])h
    out = y @ Wp.T + bp

Sharding: core = (b, hp) with b = batch, hp = head pair (v0 dims
[128hp, 128hp+128)).  v2 design notes:
  - w[key] is folded into wvT = vT * wrep (GpSimd), so QK against wvT
    yields pre-scaled scores and va = DMA-transpose(wvT) needs no
    further scaling.
  - exp is split across ACT (exact, off-diag blocks) and DVE
    (Schraudolph bf16 bit-trick: i16 = S*A + bias, bitcast; the causal
    mask of diagonal blocks is fused via scalar_tensor_tensor with a
    bias/mask tile: masked lanes get +4000 -> bitcast ~1e-25 ~ 0).
  - QK row-tiled: the two heads' K=64 matmuls run concurrently on
    array row strips (tile_position (0,0)/(64,0)).
  - softmax denominator rides as a 65th ones-column in the AV
    stationary; 1/d via bf16 magic-seed + 1 Newton step on DVE; the
    (negated) reciprocal is partition-broadcast by GpSimd and the sign
    is folded into a host-negated Wp.
  - PSUM is hand-placed in one [128, 4096] arena: banks 0-3 = S
    ping/pong, banks 4-7 = two yps pairs (alternating qj parity), with
    V-proj and out-proj matmuls reusing freed yps banks.
Host: out[b] = sum_hp outT^T + bp  (partial-sum reduce off-device).
"""

import ml_dtypes
import numpy as np

B, T, C = 2, 2048, 512
NH, HD = 8, 64
P = 128
QB = 512                 # query chunk
NQ = 4                   # query chunks
NKB = 16                 # key blocks of 128

EXP_A = 128 * 1.4426950408889634     # ln->bf16-exponent scale
EXP_BIAS = 16250.625                 # 16256 - C (C calibrated)
MASK_BIAS = 4000.0                   # masked lanes -> tiny positive
MAGIC16 = 0x7EF3                     # bf16 reciprocal seed

_cache = {}


def _split_multi_waits(nc, mybir):
    """Walrus in this container encodes at most ONE sync wait (and one
    update) per instruction; Tile's sem assignment emits several. Hoist
    excess waits onto single-wait NOPs placed just before the
    instruction on the same engine, and excess updates of non-DMA
    instructions onto NOPs just after."""
    dma_ops = {"DMACopy", "DMATranspose", "TensorCopy"}
    for f in nc.m.functions:
        for bb in f.blocks:
            new = []
            changed = False
            for inst in bb.instructions:
                si = inst.sync_info
                waits = list(si.on_wait or []) if si is not None else []
                ups = list(si.on_update or []) if si is not None else []
                is_dma = inst.concise_opcode() in dma_ops if hasattr(
                    inst, "concise_opcode") else False
                post = []
                if si is not None and len(waits) > 1:
                    for w in waits[:-1]:
                        nop = mybir.InstNoOp(
                            name=nc.get_next_instruction_name(),
                            sync_info=mybir.SyncInfo(on_wait=[w], on_update=[]),
                            bass_nofuse=True,
                            engine=inst.engine,
                        )
                        nc.register_instruction(nop, overwrite=True)
                        new.append(nop)
                    waits = waits[-1:]
                    inst.sync_info = mybir.SyncInfo(on_wait=waits, on_update=ups)
                    changed = True
                if si is not None and len(ups) > 1 and not is_dma:
                    for u in ups[1:]:
                        nop = mybir.InstNoOp(
                            name=nc.get_next_instruction_name(),
                            sync_info=mybir.SyncInfo(on_wait=[], on_update=[u]),
                            bass_nofuse=True,
                            engine=inst.engine,
                        )
                        nc.register_instruction(nop, overwrite=True)
                        post.append(nop)
                    inst.sync_info = mybir.SyncInfo(
                        on_wait=waits, on_update=ups[:1])
                    changed = True
                new.append(inst)
                new.extend(post)
            if changed:
                bb.instructions = new


def _u2bias():
    # U2[k, j] = EXP_BIAS if j >= k else MASK_BIAS, doubled side by side
    # so a [p, 2, span] strided AP serves both heads of a diagonal block.
    s = np.arange(P)[:, None]
    j = np.arange(QB)[None, :]
    u = np.where(j >= s, EXP_BIAS, MASK_BIAS).astype(np.float32)
    return np.concatenate([u, u], axis=1)


def _build_nc(with_bias, debug=False):
    import concourse.bass as bass
    import concourse.mybir as mybir
    from concourse.tile import TileContext

    f32 = mybir.dt.float32
    bf16 = mybir.dt.bfloat16
    i16 = mybir.dt.int16
    AF = mybir.ActivationFunctionType
    ALU = mybir.AluOpType

    nc = bass.Bass()

    xTp = nc.dram_tensor("xTp", [P, 4 * T], bf16, kind="ExternalInput")
    wvp = nc.dram_tensor("wvp", [P, C], bf16, kind="ExternalInput")
    wpT = nc.dram_tensor("wpT", [P, C], bf16, kind="ExternalInput")
    wrp = nc.dram_tensor("wrp", [P, T], bf16, kind="ExternalInput")
    smf = nc.dram_tensor("smf", [P, 1], f32, kind="ExternalInput")
    outT = nc.dram_tensor("outT", [C, T], bf16, kind="ExternalOutput")
    if debug:
        dbg = {
            "dvT": nc.dram_tensor("dvT", [P, T], bf16,
                                  kind="ExternalOutput"),
            "dwvT": nc.dram_tensor("dwvT", [P, T], bf16,
                                   kind="ExternalOutput"),
            "dva": nc.dram_tensor("dva", [P, 4 * (2 * HD + 2)], bf16,
                                  kind="ExternalOutput"),
            "de": nc.dram_tensor("de", [P, 2 * QB], bf16,
                                 kind="ExternalOutput"),
            "de2": nc.dram_tensor("de2", [P, 2 * QB], bf16,
                                  kind="ExternalOutput"),
            "ddd": nc.dram_tensor("ddd", [33, QB], bf16,
                                  kind="ExternalOutput"),
            "dr1": nc.dram_tensor("dr1", [33, QB], bf16,
                                  kind="ExternalOutput"),
            "drr": nc.dram_tensor("drr", [P, QB], bf16,
                                  kind="ExternalOutput"),
            "dysb": nc.dram_tensor("dysb", [P, QB], bf16,
                                   kind="ExternalOutput"),
        }

    u2_d = nc.inline_tensor(_u2bias(), name="u2bias")
    id_d = nc.inline_tensor(np.eye(P).astype(ml_dtypes.bfloat16),
                            name="idn")
    mg_d = nc.inline_tensor(
        np.full((33, QB), MAGIC16, np.int16), name="magic16")
    pm_d = nc.inline_tensor(
        np.ones((33, HD), ml_dtypes.bfloat16), name="pmones")

    arena = nc.alloc_psum_tensor("arena", [P, 4096], f32)

    # arena column layout (each 512-col slab = one PSUM bank)
    SCOL = (0, 1024)                 # S ping/pong, [128, 1024] each
    YCOL = (2048, 3072)              # yps pair per qj parity

    with TileContext(nc) as tc:
        with (
            tc.tile_pool(name="persist", bufs=1) as pp,
            tc.tile_pool(name="stream", bufs=2) as sp,
        ):
            # ---- persistent SBUF ----
            xT_sb = pp.tile([P, 4 * T], bf16, tag="xTp")
            vT_sb = pp.tile([P, T], bf16, tag="vT")
            wvT_sb = pp.tile([P, T], bf16, tag="wvT")
            wrp_sb = pp.tile([P, T], bf16, tag="wrp")
            wv_sb = pp.tile([P, C], bf16, tag="wvp")
            wpT_sb = pp.tile([P, C], bf16, tag="wp")
            va_sb = [[pp.tile([P, HD + 1], bf16, tag=f"va{i}_{u}",
                              name=f"va{i}_{u}") for u in (0, 1)]
                     for i in range(NKB)]
            u2_sb = pp.tile([P, 2 * QB], f32, tag="u2")
            idn_sb = pp.tile([P, P], bf16, tag="idn")
            mg_sb = pp.tile([33, QB], i16, tag="mg")
            pm_sb = pp.tile([33, HD], bf16, tag="pm")
            smf_sb = pp.tile([P, 1], f32, tag="smf")
            bvc_sb = smf_sb[:, 0:1]

            # ---- prologue DMAs, split across sync + scalar queues ----
            xT3d = xTp.rearrange("p (k t) -> p k t", t=T)
            xT3s = xT_sb[:].rearrange("p (k t) -> p k t", t=T)
            nc.sync.dma_start(out=wv_sb[:], in_=wvp[:])
            for k in range(4):
                eng = nc.sync if k % 2 == 0 else nc.scalar
                eng.dma_start(out=xT3s[:, k, 0:QB], in_=xT3d[:, k, 0:QB])
            nc.scalar.dma_start(out=wrp_sb[:], in_=wrp[:])
            nc.sync.dma_start(out=u2_sb[:], in_=u2_d[:])
            nc.scalar.dma_start(out=idn_sb[:], in_=id_d[:])
            for c in range(1, 4):
                eng = nc.sync if c % 2 == 1 else nc.scalar
                eng.dma_start(out=xT3s[:, :, c * QB:(c + 1) * QB],
                              in_=xT3d[:, :, c * QB:(c + 1) * QB])
            nc.scalar.dma_start(out=wpT_sb[:], in_=wpT[:])
            nc.sync.dma_start(out=mg_sb[:], in_=mg_d[:])
            nc.scalar.dma_start(out=pm_sb[:], in_=pm_d[:])
            if with_bias:
                nc.scalar.dma_start(out=smf_sb[:], in_=smf[:])
            for i in range(NKB):
                for u in (0, 1):
                    nc.gpsimd.memset(va_sb[i][u][:, HD:HD + 1], 1.0)

            # ---- PE warm-up: ~2.6us of dummy matmuls on wv while the
            # bulk DMAs land, so real matmuls start at the 2.4GHz clock.
            wps = arena[0:P, 0:P]
            for _ in range(22):
                nc.tensor.matmul(wps, wv_sb[:, 0:P], wv_sb[:, 0:P],
                                 start=True, stop=True)

            ebuf = {}
            ysb_t = {}
            r1_t = {}
            ot_t = {}

            def ycols(qj, u):
                base = YCOL[qj % 2]
                return slice(base + u * QB, base + (u + 1) * QB)

            def emit_VP(qj):
                vps = arena[0:P, YCOL[qj % 2]:YCOL[qj % 2] + QB]
                for k in range(4):
                    nc.tensor.matmul(
                        vps, wv_sb[:, k * P:(k + 1) * P],
                        xT_sb[:, k * T + qj * QB:k * T + (qj + 1) * QB],
                        start=(k == 0), stop=(k == 3))

            def emit_VC(qj):
                vps = arena[0:P, YCOL[qj % 2]:YCOL[qj % 2] + QB]
                dst = vT_sb[:, qj * QB:(qj + 1) * QB]
                if with_bias:
                    nc.scalar.activation(dst, vps, AF.Copy, bias=bvc_sb)
                else:
                    nc.scalar.copy(dst, vps)

            def emit_WV(qj):
                sl = slice(qj * QB, (qj + 1) * QB)
                nc.gpsimd.tensor_mul(wvT_sb[:, sl], vT_sb[:, sl],
                                     wrp_sb[:, sl])

            def emit_TR(qj):
                # PE-transpose each key block of wvT into transient bf16
                # psum (free half of chunk-qj's yps pair), then evacuate
                # per-head halves into the va tiles (ACT u0 / DVE u1).
                tbase = YCOL[qj % 2] + QB
                for g in range(4):
                    kb = 4 * qj + g
                    tps = arena[0:P, tbase + g * HD:
                                tbase + (g + 1) * HD].bitcast(bf16)
                    nc.tensor.transpose(
                        tps, wvT_sb[:, kb * P:(kb + 1) * P], idn_sb[:])
                    nc.scalar.copy(va_sb[kb][0][:, 0:HD], tps[:, 0:HD])
                    nc.vector.tensor_copy(va_sb[kb][1][:, 0:HD],
                                          tps[:, HD:P])

            def emit_QK(qj, ki, sbuf_i):
                diag = ki >= 4 * qj
                so = P * (ki - 4 * qj) if diag else 0
                c0 = SCOL[sbuf_i]
                spair = arena[0:P, c0:c0 + 2 * QB]
                for u in (0, 1):
                    nc.tensor.matmul(
                        spair[:, u * QB + so:(u + 1) * QB],
                        wvT_sb[HD * u:HD * (u + 1), ki * P:(ki + 1) * P],
                        vT_sb[HD * u:HD * (u + 1),
                              qj * QB + so:(qj + 1) * QB],
                        start=True, stop=True,
                        tile_position=(HD * u, 0))
                return spair, so

            def emit_EXP(qj, ki, spair, so, eng):
                diag = ki >= 4 * qj
                e = sp.tile([P, 2 * QB], bf16, tag="e", name=f"e{qj}_{ki}",
                            bufs=4)
                if eng == "act":
                    if not diag:
                        nc.scalar.activation(e[:], spair, AF.Exp,
                                             scale=0.125)
                    else:
                        e3 = e[:].rearrange("p (u q) -> p u q", q=QB)
                        s3 = spair.rearrange("p (u q) -> p u q", q=QB)
                        nc.scalar.activation(
                            e3[:, :, so:QB], s3[:, :, so:QB], AF.Exp,
                            scale=0.125)
                        u3 = u2_sb[:].rearrange("p (u q) -> p u q", q=QB)
                        # mask on gpsimd: e *= (bias>=MASK? ) -- use
                        # 0/1 trick: compare not avail; multiply by U
                        # is handled below via DVE fallback; ACT-diag
                        # not used by default policy.
                        raise AssertionError("ACT diag not supported")
                else:
                    ei = e[:].bitcast(i16)
                    if not diag:
                        nc.vector.tensor_scalar(
                            ei, spair, EXP_A / 8.0, EXP_BIAS,
                            ALU.mult, ALU.add)
                    else:
                        e3 = ei.rearrange("p (u q) -> p u q", q=QB)
                        s3 = spair.rearrange("p (u q) -> p u q", q=QB)
                        u3 = u2_sb[:].rearrange("p (u q) -> p u q", q=QB)
                        nc.vector.scalar_tensor_tensor(
                            e3[:, :, so:QB], s3[:, :, so:QB], EXP_A / 8.0,
                            u3[:, :, 0:QB - so], ALU.mult, ALU.add)
                if debug and (qj, ki) == (0, 0):
                    nc.sync.dma_start(out=dbg["de"][:], in_=e[:])
                if debug and (qj, ki) == (1, 0):
                    nc.sync.dma_start(out=dbg["de2"][:], in_=e[:])
                ebuf[(qj, ki)] = (e, so)

            def emit_AV(qj, ki):
                e, so = ebuf.pop((qj, ki))
                last = ki == 4 * qj + 3
                for u in (0, 1):
                    nc.tensor.matmul(
                        arena[0:HD + 1, ycols(qj, u)][:, so:QB],
                        va_sb[ki][u][:],
                        e[:, u * QB + so:(u + 1) * QB],
                        start=(ki == 0), stop=last)

            def emit_DCOPY(qj):
                # d rows land on partitions 0 and 32 (legal AP bases).
                dd = sp.tile([33, QB], bf16, tag="dd", name=f"dd{qj}",
                             bufs=2)
                nc.scalar.copy(dd[0:1, :], arena[HD:HD + 1, ycols(qj, 0)])
                nc.vector.tensor_copy(dd[32:33, :],
                                      arena[HD:HD + 1, ycols(qj, 1)])
                if debug and qj == 0:
                    nc.sync.dma_start(out=dbg["ddd"][:], in_=dd[:])
                return dd

            def emit_CHAIN(qj, dd):
                # -1/d in bf16: magic seed + one Newton step.
                # r0 = bitcast(magic - bits(d)); t = d*r0; r1n = (t-2)*r0
                r0 = sp.tile([33, QB], bf16, tag="r0", name=f"r0_{qj}",
                             bufs=2)
                nc.vector.tensor_tensor(r0[:].bitcast(i16), mg_sb[:],
                                        dd[:].bitcast(i16), ALU.subtract)
                t = sp.tile([33, QB], bf16, tag="rt", name=f"rt{qj}",
                            bufs=2)
                nc.vector.tensor_mul(t[:], dd[:], r0[:])
                r1n = sp.tile([33, QB], bf16, tag="r1", name=f"r1_{qj}",
                              bufs=2)
                nc.vector.scalar_tensor_tensor(
                    r1n[:], t[:], 2.0, r0[:], ALU.subtract, ALU.mult)
                if debug and qj == 0:
                    nc.sync.dma_start(out=dbg["dr1"][:], in_=r1n[:])
                r1_t[qj] = r1n

            def emit_DP(qj):
                # broadcast -1/d across partitions with two K=1 matmuls
                # into the free bank of this chunk's (freed) yps pair.
                r1n = r1_t.pop(qj)
                dp = arena[0:P, YCOL[qj % 2] + QB:YCOL[qj % 2] + 2 * QB]
                nc.tensor.matmul(dp[0:HD, :], pm_sb[0:1, :], r1n[0:1, :],
                                 start=True, stop=True)
                nc.tensor.matmul(dp[HD:P, :], pm_sb[32:33, :],
                                 r1n[32:33, :], start=True, stop=True)
                return dp

            def emit_YRAW(qj):
                # evacuate unnormalized y early: frees the yps psum pair
                # without waiting for the reciprocal round-trip.
                yraw = sp.tile([P, QB], bf16, tag="yw", name=f"yw{qj}",
                               bufs=2)
                nc.scalar.copy(yraw[0:HD, :], arena[0:HD, ycols(qj, 0)])
                nc.vector.tensor_copy(yraw[HD:P, :],
                                      arena[0:HD, ycols(qj, 1)])
                return yraw

            def emit_YMUL(qj, yraw, dp):
                ysb = sp.tile([P, QB], bf16, tag="y", name=f"ysb{qj}",
                              bufs=2)
                nc.vector.tensor_mul(ysb[:], yraw[:], dp)
                if debug and qj == 0:
                    nc.sync.dma_start(out=dbg["dysb"][:], in_=ysb[:])
                ysb_t[qj] = ysb

            def emit_OP(qj, j):
                # out-proj c-chunk j for query chunk qj; psum reuses the
                # (qj)%2... note: emitted during chunk qj+1, whose parity
                # pair (qj%2) was freed by YN(qj).
                base = YCOL[qj % 2] + (j % 2) * QB
                ops = arena[0:P, base:base + QB]
                nc.tensor.matmul(ops, wpT_sb[:, j * P:(j + 1) * P],
                                 ysb_t[qj][:], start=True, stop=True)

            def emit_OT(qj, pair):
                # evacuate op psum pair (2j, 2j+1) as one [128,1024] copy
                base = YCOL[qj % 2]
                src = arena[0:P, base:base + 2 * QB]
                ot = sp.tile([P, 2 * QB], bf16, tag="ot",
                             name=f"ot{qj}_{pair}", bufs=2)
                if pair == 0:
                    nc.scalar.copy(ot[:], src)
                else:
                    nc.vector.tensor_copy(ot[:], src)
                ot_t[(qj, pair)] = ot

            outT3 = outT.rearrange("(k p) t -> p k t", p=P)

            def emit_OD(qj, pair):
                ot = ot_t.pop((qj, pair))
                nc.sync.dma_start(
                    out=outT3[:, 2 * pair:2 * pair + 2,
                              qj * QB:(qj + 1) * QB],
                    in_=ot[:].rearrange("p (k t) -> p k t", t=QB))

            # exp engine policy: diagonal -> DVE (fused mask); off-diag
            # mostly ACT, every 8th to DVE for balance.
            od_counter = [0]

            def exp_engine(qj, ki):
                if ki >= 4 * qj:
                    return "dve"
                od_counter[0] += 1
                return "dve" if od_counter[0] % 8 == 0 else "act"

            # ---- software-pipelined schedule ----
            emit_VP(0)
            emit_VC(0)
            emit_WV(0)
            emit_TR(0)

            def boundary_extras(pq, qj):
                """Ordered (slot, thunk) list: recip/norm/out-proj for
                chunk pq, and V-path prefetch for chunk qj+1."""
                st = {}
                ex = [
                    (1, lambda: st.__setitem__("dd", emit_DCOPY(pq))),
                    (2, lambda: emit_CHAIN(pq, st["dd"])),
                    (3, lambda: st.__setitem__("yw", emit_YRAW(pq))),
                ]
                if qj <= 2:
                    ex += [(4, lambda: emit_VP(qj + 1))]
                ex += [(4, lambda: st.__setitem__("dp", emit_DP(pq)))]
                if qj <= 2:
                    ex += [(5, lambda: (emit_VC(qj + 1), emit_WV(qj + 1)))]
                ex += [
                    (8, lambda: emit_YMUL(pq, st["yw"], st["dp"])),
                ]
                if qj <= 2:
                    ex += [(9, lambda: emit_TR(qj + 1))]
                ex += [
                    (10, lambda: emit_OP(pq, 0)),
                    (11, lambda: emit_OP(pq, 1)),
                    (12, lambda: (emit_OT(pq, 0), emit_OP(pq, 2))),
                    (13, lambda: (emit_OP(pq, 3), emit_OD(pq, 0))),
                    (14, lambda: emit_OT(pq, 1)),
                    (15, lambda: (emit_OD(pq, 1), ysb_t.pop(pq))),
                ]
                return ex

            for qj in range(NQ):
                nki = 4 * qj + 4
                if qj == 0:
                    extras = [(1, lambda: emit_VP(1)),
                              (2, lambda: (emit_VC(1), emit_WV(1))),
                              (3, lambda: emit_TR(1))]
                else:
                    extras = boundary_extras(qj - 1, qj)
                for i in range(nki + 1):
                    if i < nki:
                        spair, so = emit_QK(qj, i, i % 2)
                        emit_EXP(qj, i, spair, so, exp_engine(qj, i))
                    if 1 <= i <= nki:
                        emit_AV(qj, i - 1)
                    while extras and extras[0][0] <= i:
                        extras.pop(0)[1]()
                for _, thunk in extras:
                    thunk()

            # ---- epilogue for the last chunk ----
            for _, thunk in boundary_extras(NQ - 1, NQ - 1):
                thunk()
            if debug:
                nc.sync.dma_start(out=dbg["dvT"][:], in_=vT_sb[:])
                nc.sync.dma_start(out=dbg["dwvT"][:], in_=wvT_sb[:])
                dva3 = dbg["dva"].rearrange("p (k u c) -> p k u c",
                                            c=HD + 1, u=2)
                for kb in range(4):
                    for u in (0, 1):
                        nc.sync.dma_start(out=dva3[:, kb, u, :],
                                          in_=va_sb[kb][u][:])

    import concourse.mybir as mybir2
    _split_multi_waits(nc, mybir2)
    return nc


def _get_nc(with_bias=False, debug=False):
    key = f"nc{int(with_bias)}{int(debug)}"
    if key not in _cache:
        _cache[key] = _build_nc(with_bias, debug)
    return _cache[key]


def _make_in_maps(x, weight, Wv, bv, Wp, bp, state):
    x = np.asarray(x, np.float32)
    w = np.asarray(weight, np.float32)[:, :, 0]
    if not int(np.asarray(state)):
        w = np.ones_like(w)
    WvT = np.asarray(Wv, np.float32).T
    WpTn = -np.asarray(Wp, np.float32).T      # negated: folds -1/d sign
    bv = np.asarray(bv, np.float32)

    in_maps = []
    for core in range(8):
        b, hp = core // 4, core % 4
        js = slice(P * hp, P * (hp + 1))
        xTb = x[b].T.reshape(4, P, T).transpose(1, 0, 2).reshape(P, 4 * T)
        wvpb = WvT[:, js].reshape(4, P, P).transpose(1, 0, 2).reshape(P, C)
        wrpb = np.broadcast_to(w[b][None, :], (P, T))
        smfb = bv[js].reshape(P, 1)
        in_maps.append({
            "xTp": np.ascontiguousarray(xTb).astype(ml_dtypes.bfloat16),
            "wvp": np.ascontiguousarray(wvpb).astype(ml_dtypes.bfloat16),
            "wpT": np.ascontiguousarray(WpTn[js, :]).astype(
                ml_dtypes.bfloat16),
            "wrp": np.ascontiguousarray(wrpb).astype(ml_dtypes.bfloat16),
            "smf": np.ascontiguousarray(smfb),
        })
    return in_maps


def _gather(results, x=None, bp=None):
    out = np.empty((B, T, C), np.float32)
    for b in range(B):
        acc = np.zeros((C, T), np.float32)
        for hp in range(4):
            acc += results[4 * b + hp]["outT"].astype(np.float32)
        out[b] = acc.T
    if bp is not None:
        out += np.asarray(bp, np.float32)[None, None, :]
    return out


def _run(in_maps, with_bias=False, debug=False, **kw):
    from concourse.bass_utils import run_bass_kernel_spmd
    return run_bass_kernel_spmd(
        _get_nc(with_bias, debug), in_maps, list(range(8)), **kw)


def kernel(x, weight, Wv, bv, Wp, bp, state):
    in_maps = _make_in_maps(x, weight, Wv, bv, Wp, bp, state)
    res = _run(in_maps, with_bias=bool(np.any(np.asarray(bv))))
    return _gather(res.results, x, bp)


# revision 21
# speedup vs baseline: 1.2482x; 1.0104x over previous
"""Causal self-attention (weight-modulated) Trainium2 kernel, 8-core SPMD.

Reference semantics (B=2, T=2048, C=512, 8 heads, hd=64):
    v0  = x @ Wv.T + bv
    att = softmax(mask((v0h @ v0h^T) * w[key] / sqrt(hd)))
    y   = att @ (v0*w[row])h
    out = y @ Wp.T + bp

Sharding: core = (b, hp) with b = batch, hp = head pair (v0 dims
[128hp, 128hp+128)).  v2 design notes:
  - w[key] is folded into wvT = vT * wrep (GpSimd), so QK against wvT
    yields pre-scaled scores and va = DMA-transpose(wvT) needs no
    further scaling.
  - exp is split across ACT (exact, off-diag blocks) and DVE
    (Schraudolph bf16 bit-trick: i16 = S*A + bias, bitcast; the causal
    mask of diagonal blocks is fused via scalar_tensor_tensor with a
    bias/mask tile: masked lanes get +4000 -> bitcast ~1e-25 ~ 0).
  - QK row-tiled: the two heads' K=64 matmuls run concurrently on
    array row strips (tile_position (0,0)/(64,0)).
  - softmax denominator rides as a 65th ones-column in the AV
    stationary; 1/d via bf16 magic-seed + 1 Newton step on DVE; the
    (negated) reciprocal is partition-broadcast by GpSimd and the sign
    is folded into a host-negated Wp.
  - PSUM is hand-placed in one [128, 4096] arena: banks 0-3 = S
    ping/pong, banks 4-7 = two yps pairs (alternating qj parity), with
    V-proj and out-proj matmuls reusing freed yps banks.
Host: out[b] = sum_hp outT^T + bp  (partial-sum reduce off-device).
"""

import ml_dtypes
import numpy as np

B, T, C = 2, 2048, 512
NH, HD = 8, 64
P = 128
QB = 512                 # query chunk
NQ = 4                   # query chunks
NKB = 16                 # key blocks of 128

EXP_A = 128 * 1.4426950408889634     # ln->bf16-exponent scale
EXP_BIAS = 16250.625                 # 16256 - C (C calibrated)
MASK_BIAS = 4000.0                   # masked lanes -> tiny positive
MAGIC16 = 0x7EF3                     # bf16 reciprocal seed

_cache = {}


def _split_multi_waits(nc, mybir):
    """Walrus in this container encodes at most ONE sync wait (and one
    update) per instruction; Tile's sem assignment emits several. Hoist
    excess waits onto single-wait NOPs placed just before the
    instruction on the same engine, and excess updates of non-DMA
    instructions onto NOPs just after."""
    dma_ops = {"DMACopy", "DMATranspose", "TensorCopy"}
    for f in nc.m.functions:
        for bb in f.blocks:
            new = []
            changed = False
            for inst in bb.instructions:
                si = inst.sync_info
                waits = list(si.on_wait or []) if si is not None else []
                ups = list(si.on_update or []) if si is not None else []
                is_dma = inst.concise_opcode() in dma_ops if hasattr(
                    inst, "concise_opcode") else False
                post = []
                if si is not None and len(waits) > 1:
                    for w in waits[:-1]:
                        nop = mybir.InstNoOp(
                            name=nc.get_next_instruction_name(),
                            sync_info=mybir.SyncInfo(on_wait=[w], on_update=[]),
                            bass_nofuse=True,
                            engine=inst.engine,
                        )
                        nc.register_instruction(nop, overwrite=True)
                        new.append(nop)
                    waits = waits[-1:]
                    inst.sync_info = mybir.SyncInfo(on_wait=waits, on_update=ups)
                    changed = True
                if si is not None and len(ups) > 1 and not is_dma:
                    for u in ups[1:]:
                        nop = mybir.InstNoOp(
                            name=nc.get_next_instruction_name(),
                            sync_info=mybir.SyncInfo(on_wait=[], on_update=[u]),
                            bass_nofuse=True,
                            engine=inst.engine,
                        )
                        nc.register_instruction(nop, overwrite=True)
                        post.append(nop)
                    inst.sync_info = mybir.SyncInfo(
                        on_wait=waits, on_update=ups[:1])
                    changed = True
                new.append(inst)
                new.extend(post)
            if changed:
                bb.instructions = new


def _u2bias():
    # U2[k, j] = EXP_BIAS if j >= k else MASK_BIAS, doubled side by side
    # so a [p, 2, span] strided AP serves both heads of a diagonal block.
    s = np.arange(P)[:, None]
    j = np.arange(QB)[None, :]
    u = np.where(j >= s, EXP_BIAS, MASK_BIAS).astype(np.float32)
    return np.concatenate([u, u], axis=1)


def _build_nc(with_bias, debug=False):
    import concourse.bass as bass
    import concourse.mybir as mybir
    from concourse.tile import TileContext

    f32 = mybir.dt.float32
    bf16 = mybir.dt.bfloat16
    i16 = mybir.dt.int16
    AF = mybir.ActivationFunctionType
    ALU = mybir.AluOpType

    nc = bass.Bass()

    xTp = nc.dram_tensor("xTp", [P, 4 * T], bf16, kind="ExternalInput")
    wvp = nc.dram_tensor("wvp", [P, C], bf16, kind="ExternalInput")
    wpT = nc.dram_tensor("wpT", [P, C], bf16, kind="ExternalInput")
    wrp = nc.dram_tensor("wrp", [P, T], bf16, kind="ExternalInput")
    smf = nc.dram_tensor("smf", [P, 1], f32, kind="ExternalInput")
    outT = nc.dram_tensor("outT", [C, T], bf16, kind="ExternalOutput")
    if debug:
        dbg = {
            "dvT": nc.dram_tensor("dvT", [P, T], bf16,
                                  kind="ExternalOutput"),
            "dwvT": nc.dram_tensor("dwvT", [P, T], bf16,
                                   kind="ExternalOutput"),
            "dva": nc.dram_tensor("dva", [P, 4 * (2 * HD + 2)], bf16,
                                  kind="ExternalOutput"),
            "de": nc.dram_tensor("de", [P, 2 * QB], bf16,
                                 kind="ExternalOutput"),
            "de2": nc.dram_tensor("de2", [P, 2 * QB], bf16,
                                  kind="ExternalOutput"),
            "ddd": nc.dram_tensor("ddd", [33, QB], bf16,
                                  kind="ExternalOutput"),
            "dr1": nc.dram_tensor("dr1", [33, QB], bf16,
                                  kind="ExternalOutput"),
            "drr": nc.dram_tensor("drr", [P, QB], bf16,
                                  kind="ExternalOutput"),
            "dysb": nc.dram_tensor("dysb", [P, QB], bf16,
                                   kind="ExternalOutput"),
        }

    u2_d = nc.inline_tensor(_u2bias(), name="u2bias")
    id_d = nc.inline_tensor(np.eye(P).astype(ml_dtypes.bfloat16),
                            name="idn")
    mg_d = nc.inline_tensor(
        np.full((33, QB), MAGIC16, np.int16), name="magic16")
    pm_d = nc.inline_tensor(
        np.ones((33, HD), ml_dtypes.bfloat16), name="pmones")

    arena = nc.alloc_psum_tensor("arena", [P, 4096], f32)

    # arena column layout (each 512-col slab = one PSUM bank)
    SCOL = (0, 1024)                 # S ping/pong, [128, 1024] each
    YCOL = (2048, 3072)              # yps pair per qj parity

    with TileContext(nc) as tc:
        with (
            tc.tile_pool(name="persist", bufs=1) as pp,
            tc.tile_pool(name="stream", bufs=2) as sp,
        ):
            # ---- persistent SBUF ----
            xT_sb = pp.tile([P, 4 * T], bf16, tag="xTp")
            vT_sb = pp.tile([P, T], bf16, tag="vT")
            wvT_sb = pp.tile([P, T], bf16, tag="wvT")
            wrp_sb = pp.tile([P, T], bf16, tag="wrp")
            wv_sb = pp.tile([P, C], bf16, tag="wvp")
            wpT_sb = pp.tile([P, C], bf16, tag="wp")
            va_sb = [[pp.tile([P, HD + 1], bf16, tag=f"va{i}_{u}",
                              name=f"va{i}_{u}") for u in (0, 1)]
                     for i in range(NKB)]
            u2_sb = pp.tile([P, 2 * QB], f32, tag="u2")
            idn_sb = pp.tile([P, P], bf16, tag="idn")
            mg_sb = pp.tile([33, QB], i16, tag="mg")
            pm_sb = pp.tile([33, HD], bf16, tag="pm")
            smf_sb = pp.tile([P, 1], f32, tag="smf")
            bvc_sb = smf_sb[:, 0:1]

            # ---- prologue DMAs, split across sync + scalar queues ----
            xT3d = xTp.rearrange("p (k t) -> p k t", t=T)
            xT3s = xT_sb[:].rearrange("p (k t) -> p k t", t=T)
            nc.scalar.dma_start(out=idn_sb[:], in_=id_d[:])
            for k in range(4):
                eng = nc.sync if k % 2 == 0 else nc.scalar
                eng.dma_start(out=xT3s[:, k, 0:QB], in_=xT3d[:, k, 0:QB])
            nc.sync.dma_start(out=wv_sb[:], in_=wvp[:])
            nc.scalar.dma_start(out=wrp_sb[:], in_=wrp[:])
            nc.sync.dma_start(out=u2_sb[:], in_=u2_d[:])
            for c in range(1, 4):
                eng = nc.sync if c % 2 == 1 else nc.scalar
                eng.dma_start(out=xT3s[:, :, c * QB:(c + 1) * QB],
                              in_=xT3d[:, :, c * QB:(c + 1) * QB])
            nc.scalar.dma_start(out=wpT_sb[:], in_=wpT[:])
            nc.sync.dma_start(out=mg_sb[:], in_=mg_d[:])
            nc.scalar.dma_start(out=pm_sb[:], in_=pm_d[:])
            if with_bias:
                nc.scalar.dma_start(out=smf_sb[:], in_=smf[:])
            for i in range(NKB):
                for u in (0, 1):
                    nc.gpsimd.memset(va_sb[i][u][:, HD:HD + 1], 1.0)

            # ---- PE warm-up: ~2.6us of dummy matmuls on wv while the
            # bulk DMAs land, so real matmuls start at the 2.4GHz clock.
            wps = arena[0:P, 0:P]
            for _ in range(24):
                nc.tensor.matmul(wps, idn_sb[:], idn_sb[:],
                                 start=True, stop=True)

            ebuf = {}
            ysb_t = {}
            r1_t = {}
            ot_t = {}

            def ycols(qj, u):
                base = YCOL[qj % 2]
                return slice(base + u * QB, base + (u + 1) * QB)

            def emit_VP(qj):
                vps = arena[0:P, YCOL[qj % 2]:YCOL[qj % 2] + QB]
                for k in range(4):
                    nc.tensor.matmul(
                        vps, wv_sb[:, k * P:(k + 1) * P],
                        xT_sb[:, k * T + qj * QB:k * T + (qj + 1) * QB],
                        start=(k == 0), stop=(k == 3))

            def emit_VC(qj):
                vps = arena[0:P, YCOL[qj % 2]:YCOL[qj % 2] + QB]
                dst = vT_sb[:, qj * QB:(qj + 1) * QB]
                if with_bias:
                    nc.scalar.activation(dst, vps, AF.Copy, bias=bvc_sb)
                else:
                    nc.scalar.copy(dst, vps)

            def emit_WV(qj):
                sl = slice(qj * QB, (qj + 1) * QB)
                nc.gpsimd.tensor_mul(wvT_sb[:, sl], vT_sb[:, sl],
                                     wrp_sb[:, sl])

            def emit_TR(qj):
                # PE-transpose each key block of wvT into transient bf16
                # psum (free half of chunk-qj's yps pair), then evacuate
                # per-head halves into the va tiles (ACT u0 / DVE u1).
                tbase = YCOL[qj % 2] + QB
                for g in range(4):
                    kb = 4 * qj + g
                    tps = arena[0:P, tbase + g * HD:
                                tbase + (g + 1) * HD].bitcast(bf16)
                    nc.tensor.transpose(
                        tps, wvT_sb[:, kb * P:(kb + 1) * P], idn_sb[:])
                    nc.vector.tensor_copy(va_sb[kb][0][:, 0:HD],
                                          tps[:, 0:HD])
                    nc.vector.tensor_copy(va_sb[kb][1][:, 0:HD],
                                          tps[:, HD:P])

            def emit_QK(qj, ki, sbuf_i):
                diag = ki >= 4 * qj
                so = P * (ki - 4 * qj) if diag else 0
                c0 = SCOL[sbuf_i]
                spair = arena[0:P, c0:c0 + 2 * QB]
                for u in (0, 1):
                    nc.tensor.matmul(
                        spair[:, u * QB + so:(u + 1) * QB],
                        wvT_sb[HD * u:HD * (u + 1), ki * P:(ki + 1) * P],
                        vT_sb[HD * u:HD * (u + 1),
                              qj * QB + so:(qj + 1) * QB],
                        start=True, stop=True,
                        tile_position=(HD * u, 0))
                return spair, so

            def emit_EXP(qj, ki, spair, so, eng):
                diag = ki >= 4 * qj
                e = sp.tile([P, 2 * QB], bf16, tag="e", name=f"e{qj}_{ki}",
                            bufs=6)
                if eng == "act":
                    if not diag:
                        nc.scalar.activation(e[:], spair, AF.Exp,
                                             scale=0.125)
                    else:
                        e3 = e[:].rearrange("p (u q) -> p u q", q=QB)
                        s3 = spair.rearrange("p (u q) -> p u q", q=QB)
                        nc.scalar.activation(
                            e3[:, :, so:QB], s3[:, :, so:QB], AF.Exp,
                            scale=0.125)
                        u3 = u2_sb[:].rearrange("p (u q) -> p u q", q=QB)
                        # mask on gpsimd: e *= (bias>=MASK? ) -- use
                        # 0/1 trick: compare not avail; multiply by U
                        # is handled below via DVE fallback; ACT-diag
                        # not used by default policy.
                        raise AssertionError("ACT diag not supported")
                else:
                    ei = e[:].bitcast(i16)
                    if not diag:
                        nc.vector.tensor_scalar(
                            ei, spair, EXP_A / 8.0, EXP_BIAS,
                            ALU.mult, ALU.add)
                    else:
                        e3 = ei.rearrange("p (u q) -> p u q", q=QB)
                        s3 = spair.rearrange("p (u q) -> p u q", q=QB)
                        u3 = u2_sb[:].rearrange("p (u q) -> p u q", q=QB)
                        nc.vector.scalar_tensor_tensor(
                            e3[:, :, so:QB], s3[:, :, so:QB], EXP_A / 8.0,
                            u3[:, :, 0:QB - so], ALU.mult, ALU.add)
                if debug and (qj, ki) == (0, 0):
                    nc.sync.dma_start(out=dbg["de"][:], in_=e[:])
                if debug and (qj, ki) == (1, 0):
                    nc.sync.dma_start(out=dbg["de2"][:], in_=e[:])
                ebuf[(qj, ki)] = (e, so)

            def emit_AV(qj, ki):
                e, so = ebuf.pop((qj, ki))
                last = ki == 4 * qj + 3
                for u in (0, 1):
                    nc.tensor.matmul(
                        arena[0:HD + 1, ycols(qj, u)][:, so:QB],
                        va_sb[ki][u][:],
                        e[:, u * QB + so:(u + 1) * QB],
                        start=(ki == 0), stop=last)

            def emit_DCOPY(qj):
                # d rows land on partitions 0 and 32 (legal AP bases).
                dd = sp.tile([33, QB], bf16, tag="dd", name=f"dd{qj}",
                             bufs=2)
                nc.scalar.copy(dd[0:1, :], arena[HD:HD + 1, ycols(qj, 0)])
                nc.vector.tensor_copy(dd[32:33, :],
                                      arena[HD:HD + 1, ycols(qj, 1)])
                if debug and qj == 0:
                    nc.sync.dma_start(out=dbg["ddd"][:], in_=dd[:])
                return dd

            def emit_CHAIN(qj, dd):
                # -1/d in bf16: magic seed + one Newton step.
                # r0 = bitcast(magic - bits(d)); t = d*r0; r1n = (t-2)*r0
                r0 = sp.tile([33, QB], bf16, tag="r0", name=f"r0_{qj}",
                             bufs=2)
                nc.vector.tensor_tensor(r0[:].bitcast(i16), mg_sb[:],
                                        dd[:].bitcast(i16), ALU.subtract)
                t = sp.tile([33, QB], bf16, tag="rt", name=f"rt{qj}",
                            bufs=2)
                nc.vector.tensor_mul(t[:], dd[:], r0[:])
                r1n = sp.tile([33, QB], bf16, tag="r1", name=f"r1_{qj}",
                              bufs=2)
                nc.vector.scalar_tensor_tensor(
                    r1n[:], t[:], 2.0, r0[:], ALU.subtract, ALU.mult)
                if debug and qj == 0:
                    nc.sync.dma_start(out=dbg["dr1"][:], in_=r1n[:])
                r1_t[qj] = r1n

            def emit_DP(qj):
                # broadcast -1/d across partitions with two K=1 matmuls
                # into the free bank of this chunk's (freed) yps pair.
                r1n = r1_t.pop(qj)
                dp = arena[0:P, YCOL[qj % 2] + QB:YCOL[qj % 2] + 2 * QB]
                nc.tensor.matmul(dp[0:HD, :], pm_sb[0:1, :], r1n[0:1, :],
                                 start=True, stop=True)
                nc.tensor.matmul(dp[HD:P, :], pm_sb[32:33, :],
                                 r1n[32:33, :], start=True, stop=True)
                return dp

            def emit_YRAW(qj):
                # evacuate unnormalized y early: frees the yps psum pair
                # without waiting for the reciprocal round-trip.
                yraw = sp.tile([P, QB], bf16, tag="yw", name=f"yw{qj}",
                               bufs=2)
                nc.scalar.copy(yraw[0:HD, :], arena[0:HD, ycols(qj, 0)])
                nc.vector.tensor_copy(yraw[HD:P, :],
                                      arena[0:HD, ycols(qj, 1)])
                return yraw

            def emit_YMUL(qj, yraw, dp):
                ysb = sp.tile([P, QB], bf16, tag="y", name=f"ysb{qj}",
                              bufs=2)
                nc.vector.tensor_mul(ysb[:], yraw[:], dp)
                if debug and qj == 0:
                    nc.sync.dma_start(out=dbg["dysb"][:], in_=ysb[:])
                ysb_t[qj] = ysb

            def emit_OP(qj, j, sfree=False):
                # out-proj c-chunk j; psum reuses the freed yps pair of
                # parity qj%2, or all four S banks in the epilogue.
                if sfree:
                    base = j * QB
                else:
                    base = YCOL[qj % 2] + (j % 2) * QB
                ops = arena[0:P, base:base + QB]
                nc.tensor.matmul(ops, wpT_sb[:, j * P:(j + 1) * P],
                                 ysb_t[qj][:], start=True, stop=True)

            def emit_OT(qj, pair, sfree=False):
                # evacuate op psum pair (2j, 2j+1) as one [128,1024] copy
                base = (2 * QB * pair) if sfree else YCOL[qj % 2]
                src = arena[0:P, base:base + 2 * QB]
                ot = sp.tile([P, 2 * QB], bf16, tag="ot",
                             name=f"ot{qj}_{pair}", bufs=2)
                if pair == 0:
                    nc.scalar.copy(ot[:], src)
                else:
                    nc.vector.tensor_copy(ot[:], src)
                ot_t[(qj, pair)] = ot

            outT3 = outT.rearrange("(k p) t -> p k t", p=P)

            def emit_OD(qj, pair):
                ot = ot_t.pop((qj, pair))
                nc.sync.dma_start(
                    out=outT3[:, 2 * pair:2 * pair + 2,
                              qj * QB:(qj + 1) * QB],
                    in_=ot[:].rearrange("p (k t) -> p k t", t=QB))

            # exp engine policy: diagonal -> DVE (fused mask); off-diag
            # mostly ACT, every 8th to DVE for balance.
            od_counter = [0]

            def exp_engine(qj, ki):
                if ki >= 4 * qj:
                    return "dve"
                od_counter[0] += 1
                return "dve" if od_counter[0] % 8 == 0 else "act"

            # ---- software-pipelined schedule ----
            emit_VP(0)
            vps0 = arena[0:P, YCOL[0]:YCOL[0] + QB]
            for h in range(2):
                sl = slice(h * (QB // 2), (h + 1) * (QB // 2))
                if with_bias:
                    nc.scalar.activation(vT_sb[:, sl], vps0[:, sl],
                                         AF.Copy, bias=bvc_sb)
                else:
                    nc.scalar.copy(vT_sb[:, sl], vps0[:, sl])
                for b in range(2):
                    sb = slice(h * (QB // 2) + b * P,
                               h * (QB // 2) + (b + 1) * P)
                    nc.gpsimd.tensor_mul(wvT_sb[:, sb], vT_sb[:, sb],
                                         wrp_sb[:, sb])
            emit_TR(0)

            def boundary_extras(pq, qj):
                """Ordered (slot, thunk) list: recip/norm/out-proj for
                chunk pq, and V-path prefetch for chunk qj+1."""
                st = {}
                ex = [
                    (1, lambda: st.__setitem__("dd", emit_DCOPY(pq))),
                    (2, lambda: emit_CHAIN(pq, st["dd"])),
                    (3, lambda: st.__setitem__("yw", emit_YRAW(pq))),
                ]
                if qj <= 2:
                    ex += [(4, lambda: emit_VP(qj + 1))]
                ex += [(4, lambda: st.__setitem__("dp", emit_DP(pq)))]
                if qj <= 2:
                    ex += [(5, lambda: (emit_VC(qj + 1), emit_WV(qj + 1)))]
                ex += [
                    (8, lambda: emit_YMUL(pq, st["yw"], st["dp"])),
                ]
                if qj <= 2:
                    ex += [(9, lambda: emit_TR(qj + 1))]
                ex += [
                    (10, lambda: emit_OP(pq, 0)),
                    (11, lambda: emit_OP(pq, 1)),
                    (12, lambda: (emit_OT(pq, 0), emit_OP(pq, 2))),
                    (13, lambda: (emit_OP(pq, 3), emit_OD(pq, 0))),
                    (14, lambda: emit_OT(pq, 1)),
                    (15, lambda: (emit_OD(pq, 1), ysb_t.pop(pq))),
                ]
                return ex

            for qj in range(NQ):
                nki = 4 * qj + 4
                if qj == 0:
                    extras = [(1, lambda: emit_VP(1)),
                              (2, lambda: (emit_VC(1), emit_WV(1))),
                              (3, lambda: emit_TR(1))]
                else:
                    extras = boundary_extras(qj - 1, qj)
                for i in range(nki + 1):
                    if i < nki:
                        spair, so = emit_QK(qj, i, i % 2)
                        emit_EXP(qj, i, spair, so, exp_engine(qj, i))
                    if 1 <= i <= nki:
                        emit_AV(qj, i - 1)
                    while extras and extras[0][0] <= i:
                        extras.pop(0)[1]()
                for _, thunk in extras:
                    thunk()

            # ---- epilogue for the last chunk: all four S banks are
            # free, so the out-proj runs without OT-gating.
            pq = NQ - 1
            dd = emit_DCOPY(pq)
            emit_CHAIN(pq, dd)
            yw = emit_YRAW(pq)
            dp = emit_DP(pq)
            emit_YMUL(pq, yw, dp)
            for j in range(4):
                emit_OP(pq, j, sfree=True)
            emit_OT(pq, 0, sfree=True)
            emit_OT(pq, 1, sfree=True)
            emit_OD(pq, 0)
            emit_OD(pq, 1)
            ysb_t.pop(pq)
            if debug:
                nc.sync.dma_start(out=dbg["dvT"][:], in_=vT_sb[:])
                nc.sync.dma_start(out=dbg["dwvT"][:], in_=wvT_sb[:])
                dva3 = dbg["dva"].rearrange("p (k u c) -> p k u c",
                                            c=HD + 1, u=2)
                for kb in range(4):
                    for u in (0, 1):
                        nc.sync.dma_start(out=dva3[:, kb, u, :],
                                          in_=va_sb[kb][u][:])

    import concourse.mybir as mybir2
    _split_multi_waits(nc, mybir2)
    return nc


def _get_nc(with_bias=False, debug=False):
    key = f"nc{int(with_bias)}{int(debug)}"
    if key not in _cache:
        _cache[key] = _build_nc(with_bias, debug)
    return _cache[key]


def _make_in_maps(x, weight, Wv, bv, Wp, bp, state):
    x = np.asarray(x, np.float32)
    w = np.asarray(weight, np.float32)[:, :, 0]
    if not int(np.asarray(state)):
        w = np.ones_like(w)
    WvT = np.asarray(Wv, np.float32).T
    WpTn = -np.asarray(Wp, np.float32).T      # negated: folds -1/d sign
    bv = np.asarray(bv, np.float32)

    in_maps = []
    for core in range(8):
        b, hp = core // 4, core % 4
        js = slice(P * hp, P * (hp + 1))
        xTb = x[b].T.reshape(4, P, T).transpose(1, 0, 2).reshape(P, 4 * T)
        wvpb = WvT[:, js].reshape(4, P, P).transpose(1, 0, 2).reshape(P, C)
        wrpb = np.broadcast_to(w[b][None, :], (P, T))
        smfb = bv[js].reshape(P, 1)
        in_maps.append({
            "xTp": np.ascontiguousarray(xTb).astype(ml_dtypes.bfloat16),
            "wvp": np.ascontiguousarray(wvpb).astype(ml_dtypes.bfloat16),
            "wpT": np.ascontiguousarray(WpTn[js, :]).astype(
                ml_dtypes.bfloat16),
            "wrp": np.ascontiguousarray(wrpb).astype(ml_dtypes.bfloat16),
            "smf": np.ascontiguousarray(smfb),
        })
    return in_maps


def _gather(results, x=None, bp=None):
    out = np.empty((B, T, C), np.float32)
    for b in range(B):
        acc = np.zeros((C, T), np.float32)
        for hp in range(4):
            acc += results[4 * b + hp]["outT"].astype(np.float32)
        out[b] = acc.T
    if bp is not None:
        out += np.asarray(bp, np.float32)[None, None, :]
    return out


def _run(in_maps, with_bias=False, debug=False, **kw):
    from concourse.bass_utils import run_bass_kernel_spmd
    return run_bass_kernel_spmd(
        _get_nc(with_bias, debug), in_maps, list(range(8)), **kw)


def kernel(x, weight, Wv, bv, Wp, bp, state):
    in_maps = _make_in_maps(x, weight, Wv, bv, Wp, bp, state)
    res = _run(in_maps, with_bias=bool(np.any(np.asarray(bv))))
    return _gather(res.results, x, bp)


# revision 22
# speedup vs baseline: 1.2955x; 1.0379x over previous
"""Causal self-attention (weight-modulated) Trainium2 kernel, 8-core SPMD.

Reference semantics (B=2, T=2048, C=512, 8 heads, hd=64):
    v0  = x @ Wv.T + bv
    att = softmax(mask((v0h @ v0h^T) * w[key] / sqrt(hd)))
    y   = att @ (v0*w[row])h
    out = y @ Wp.T + bp

Sharding: core = (b, hp) with b = batch, hp = head pair (v0 dims
[128hp, 128hp+128)).  v2 design notes:
  - w[key] is folded into wvT = vT * wrep (GpSimd), so QK against wvT
    yields pre-scaled scores and va = DMA-transpose(wvT) needs no
    further scaling.
  - exp is split across ACT (exact, off-diag blocks) and DVE
    (Schraudolph bf16 bit-trick: i16 = S*A + bias, bitcast; the causal
    mask of diagonal blocks is fused via scalar_tensor_tensor with a
    bias/mask tile: masked lanes get +4000 -> bitcast ~1e-25 ~ 0).
  - QK row-tiled: the two heads' K=64 matmuls run concurrently on
    array row strips (tile_position (0,0)/(64,0)).
  - softmax denominator rides as a 65th ones-column in the AV
    stationary; 1/d via bf16 magic-seed + 1 Newton step on DVE; the
    (negated) reciprocal is partition-broadcast by GpSimd and the sign
    is folded into a host-negated Wp.
  - PSUM is hand-placed in one [128, 4096] arena: banks 0-3 = S
    ping/pong, banks 4-7 = two yps pairs (alternating qj parity), with
    V-proj and out-proj matmuls reusing freed yps banks.
Host: out[b] = sum_hp outT^T + bp  (partial-sum reduce off-device).
"""

import ml_dtypes
import numpy as np

B, T, C = 2, 2048, 512
NH, HD = 8, 64
P = 128
QB = 512                 # query chunk
NQ = 4                   # query chunks
NKB = 16                 # key blocks of 128

EXP_A = 128 * 1.4426950408889634     # ln->bf16-exponent scale
EXP_BIAS = 16250.625                 # 16256 - C (C calibrated)
MASK_BIAS = 4000.0                   # masked lanes -> tiny positive
MAGIC16 = 0x7EF3                     # bf16 reciprocal seed

_cache = {}


def _split_multi_waits(nc, mybir):
    """Walrus in this container encodes at most ONE sync wait (and one
    update) per instruction; Tile's sem assignment emits several. Hoist
    excess waits onto single-wait NOPs placed just before the
    instruction on the same engine, and excess updates of non-DMA
    instructions onto NOPs just after."""
    dma_ops = {"DMACopy", "DMATranspose", "TensorCopy"}
    for f in nc.m.functions:
        for bb in f.blocks:
            new = []
            changed = False
            for inst in bb.instructions:
                si = inst.sync_info
                waits = list(si.on_wait or []) if si is not None else []
                ups = list(si.on_update or []) if si is not None else []
                is_dma = inst.concise_opcode() in dma_ops if hasattr(
                    inst, "concise_opcode") else False
                post = []
                if si is not None and len(waits) > 1:
                    for w in waits[:-1]:
                        nop = mybir.InstNoOp(
                            name=nc.get_next_instruction_name(),
                            sync_info=mybir.SyncInfo(on_wait=[w], on_update=[]),
                            bass_nofuse=True,
                            engine=inst.engine,
                        )
                        nc.register_instruction(nop, overwrite=True)
                        new.append(nop)
                    waits = waits[-1:]
                    inst.sync_info = mybir.SyncInfo(on_wait=waits, on_update=ups)
                    changed = True
                if si is not None and len(ups) > 1 and not is_dma:
                    for u in ups[1:]:
                        nop = mybir.InstNoOp(
                            name=nc.get_next_instruction_name(),
                            sync_info=mybir.SyncInfo(on_wait=[], on_update=[u]),
                            bass_nofuse=True,
                            engine=inst.engine,
                        )
                        nc.register_instruction(nop, overwrite=True)
                        post.append(nop)
                    inst.sync_info = mybir.SyncInfo(
                        on_wait=waits, on_update=ups[:1])
                    changed = True
                new.append(inst)
                new.extend(post)
            if changed:
                bb.instructions = new


def _u2bias():
    # U2[k, j] = EXP_BIAS if j >= k else MASK_BIAS, doubled side by side
    # so a [p, 2, span] strided AP serves both heads of a diagonal block.
    s = np.arange(P)[:, None]
    j = np.arange(QB)[None, :]
    u = np.where(j >= s, EXP_BIAS, MASK_BIAS).astype(np.float32)
    return np.concatenate([u, u], axis=1)


def _build_nc(with_bias, debug=False):
    import concourse.bass as bass
    import concourse.mybir as mybir
    from concourse.tile import TileContext

    f32 = mybir.dt.float32
    bf16 = mybir.dt.bfloat16
    i16 = mybir.dt.int16
    AF = mybir.ActivationFunctionType
    ALU = mybir.AluOpType

    nc = bass.Bass()

    xTp = nc.dram_tensor("xTp", [P, 4 * T], bf16, kind="ExternalInput")
    wvp = nc.dram_tensor("wvp", [P, C], bf16, kind="ExternalInput")
    wpT = nc.dram_tensor("wpT", [P, C], bf16, kind="ExternalInput")
    wrp = nc.dram_tensor("wrp", [P, T], bf16, kind="ExternalInput")
    smf = nc.dram_tensor("smf", [P, 1], f32, kind="ExternalInput")
    outT = nc.dram_tensor("outT", [C, T], bf16, kind="ExternalOutput")
    if debug:
        dbg = {
            "dvT": nc.dram_tensor("dvT", [P, T], bf16,
                                  kind="ExternalOutput"),
            "dwvT": nc.dram_tensor("dwvT", [P, T], bf16,
                                   kind="ExternalOutput"),
            "dva": nc.dram_tensor("dva", [P, 4 * (2 * HD + 2)], bf16,
                                  kind="ExternalOutput"),
            "de": nc.dram_tensor("de", [P, 2 * QB], bf16,
                                 kind="ExternalOutput"),
            "de2": nc.dram_tensor("de2", [P, 2 * QB], bf16,
                                  kind="ExternalOutput"),
            "ddd": nc.dram_tensor("ddd", [33, QB], bf16,
                                  kind="ExternalOutput"),
            "dr1": nc.dram_tensor("dr1", [33, QB], bf16,
                                  kind="ExternalOutput"),
            "drr": nc.dram_tensor("drr", [P, QB], bf16,
                                  kind="ExternalOutput"),
            "dysb": nc.dram_tensor("dysb", [P, QB], bf16,
                                   kind="ExternalOutput"),
        }

    idn = nc.dram_tensor("idn", [P, P], bf16, kind="ExternalInput")
    u2_d = nc.inline_tensor(_u2bias(), name="u2bias")
    mg_d = nc.inline_tensor(
        np.full((33, QB), MAGIC16, np.int16), name="magic16")
    pm_d = nc.inline_tensor(
        np.ones((33, HD), ml_dtypes.bfloat16), name="pmones")

    arena = nc.alloc_psum_tensor("arena", [P, 4096], f32)

    # arena column layout (each 512-col slab = one PSUM bank)
    SCOL = (0, 1024)                 # S ping/pong, [128, 1024] each
    YCOL = (2048, 3072)              # yps pair per qj parity

    with TileContext(nc) as tc:
        with (
            tc.tile_pool(name="persist", bufs=1) as pp,
            tc.tile_pool(name="stream", bufs=2) as sp,
        ):
            # ---- persistent SBUF ----
            xT_sb = pp.tile([P, 4 * T], bf16, tag="xTp")
            vT_sb = pp.tile([P, T], bf16, tag="vT")
            wvT_sb = pp.tile([P, T], bf16, tag="wvT")
            wrp_sb = pp.tile([P, T], bf16, tag="wrp")
            wv_sb = pp.tile([P, C], bf16, tag="wvp")
            wpT_sb = pp.tile([P, C], bf16, tag="wp")
            va_sb = [[pp.tile([P, HD + 1], bf16, tag=f"va{i}_{u}",
                              name=f"va{i}_{u}") for u in (0, 1)]
                     for i in range(NKB)]
            u2_sb = pp.tile([P, 2 * QB], f32, tag="u2")
            idn_sb = pp.tile([P, P], bf16, tag="idn")
            mg_sb = pp.tile([33, QB], i16, tag="mg")
            pm_sb = pp.tile([33, HD], bf16, tag="pm")
            smf_sb = pp.tile([P, 1], f32, tag="smf")
            bvc_sb = smf_sb[:, 0:1]

            # ---- prologue DMAs, split across sync + scalar queues ----
            xT3d = xTp.rearrange("p (k t) -> p k t", t=T)
            xT3s = xT_sb[:].rearrange("p (k t) -> p k t", t=T)
            nc.scalar.dma_start(out=idn_sb[:], in_=idn[:])
            for k in range(4):
                eng = nc.sync if k % 2 == 0 else nc.scalar
                eng.dma_start(out=xT3s[:, k, 0:QB], in_=xT3d[:, k, 0:QB])
            nc.sync.dma_start(out=wv_sb[:], in_=wvp[:])
            nc.scalar.dma_start(out=wrp_sb[:], in_=wrp[:])
            nc.sync.dma_start(out=u2_sb[:], in_=u2_d[:])
            for c in range(1, 4):
                eng = nc.sync if c % 2 == 1 else nc.scalar
                eng.dma_start(out=xT3s[:, :, c * QB:(c + 1) * QB],
                              in_=xT3d[:, :, c * QB:(c + 1) * QB])
            nc.scalar.dma_start(out=wpT_sb[:], in_=wpT[:])
            nc.sync.dma_start(out=mg_sb[:], in_=mg_d[:])
            nc.scalar.dma_start(out=pm_sb[:], in_=pm_d[:])
            if with_bias:
                nc.scalar.dma_start(out=smf_sb[:], in_=smf[:])
            for i in range(NKB):
                for u in (0, 1):
                    nc.gpsimd.memset(va_sb[i][u][:, HD:HD + 1], 1.0)

            # ---- warm-ups while the bulk DMAs land: dummy matmuls
            # hold the PE at the 2.4GHz clock; a tiny activation absorbs
            # the ~2.7us EXP table load before the first real exp.
            wact = sp.tile([1, 8], bf16, tag="wact", bufs=1)
            nc.scalar.activation(wact[:], idn_sb[0:1, 0:8], AF.Exp,
                                 scale=1.0)
            wps = arena[0:P, 0:P]
            for _ in range(24):
                nc.tensor.matmul(wps, idn_sb[:], idn_sb[:],
                                 start=True, stop=True)

            ebuf = {}
            ysb_t = {}
            r1_t = {}
            ot_t = {}

            def ycols(qj, u):
                base = YCOL[qj % 2]
                return slice(base + u * QB, base + (u + 1) * QB)

            def emit_VP(qj):
                vps = arena[0:P, YCOL[qj % 2]:YCOL[qj % 2] + QB]
                for k in range(4):
                    nc.tensor.matmul(
                        vps, wv_sb[:, k * P:(k + 1) * P],
                        xT_sb[:, k * T + qj * QB:k * T + (qj + 1) * QB],
                        start=(k == 0), stop=(k == 3))

            def emit_VC(qj):
                vps = arena[0:P, YCOL[qj % 2]:YCOL[qj % 2] + QB]
                dst = vT_sb[:, qj * QB:(qj + 1) * QB]
                if with_bias:
                    nc.scalar.activation(dst, vps, AF.Copy, bias=bvc_sb)
                else:
                    nc.scalar.copy(dst, vps)

            def emit_WV(qj):
                sl = slice(qj * QB, (qj + 1) * QB)
                nc.gpsimd.tensor_mul(wvT_sb[:, sl], vT_sb[:, sl],
                                     wrp_sb[:, sl])

            def emit_TR(qj):
                # PE-transpose each key block of wvT into transient bf16
                # psum (free half of chunk-qj's yps pair), then evacuate
                # per-head halves into the va tiles (ACT u0 / DVE u1).
                tbase = YCOL[qj % 2] + QB
                for g in range(4):
                    kb = 4 * qj + g
                    tps = arena[0:P, tbase + g * HD:
                                tbase + (g + 1) * HD].bitcast(bf16)
                    nc.tensor.transpose(
                        tps, wvT_sb[:, kb * P:(kb + 1) * P], idn_sb[:])
                    nc.vector.tensor_copy(va_sb[kb][0][:, 0:HD],
                                          tps[:, 0:HD])
                    nc.vector.tensor_copy(va_sb[kb][1][:, 0:HD],
                                          tps[:, HD:P])

            def emit_QK(qj, ki, sbuf_i):
                diag = ki >= 4 * qj
                so = P * (ki - 4 * qj) if diag else 0
                c0 = SCOL[sbuf_i]
                spair = arena[0:P, c0:c0 + 2 * QB]
                for u in (0, 1):
                    nc.tensor.matmul(
                        spair[:, u * QB + so:(u + 1) * QB],
                        wvT_sb[HD * u:HD * (u + 1), ki * P:(ki + 1) * P],
                        vT_sb[HD * u:HD * (u + 1),
                              qj * QB + so:(qj + 1) * QB],
                        start=True, stop=True,
                        tile_position=(HD * u, 0))
                return spair, so

            def emit_EXP(qj, ki, spair, so, eng):
                diag = ki >= 4 * qj
                e = sp.tile([P, 2 * QB], bf16, tag="e", name=f"e{qj}_{ki}",
                            bufs=6)
                if eng == "act":
                    if not diag:
                        nc.scalar.activation(e[:], spair, AF.Exp,
                                             scale=0.125)
                    else:
                        e3 = e[:].rearrange("p (u q) -> p u q", q=QB)
                        s3 = spair.rearrange("p (u q) -> p u q", q=QB)
                        nc.scalar.activation(
                            e3[:, :, so:QB], s3[:, :, so:QB], AF.Exp,
                            scale=0.125)
                        u3 = u2_sb[:].rearrange("p (u q) -> p u q", q=QB)
                        # mask on gpsimd: e *= (bias>=MASK? ) -- use
                        # 0/1 trick: compare not avail; multiply by U
                        # is handled below via DVE fallback; ACT-diag
                        # not used by default policy.
                        raise AssertionError("ACT diag not supported")
                else:
                    ei = e[:].bitcast(i16)
                    if not diag:
                        nc.vector.tensor_scalar(
                            ei, spair, EXP_A / 8.0, EXP_BIAS,
                            ALU.mult, ALU.add)
                    else:
                        e3 = ei.rearrange("p (u q) -> p u q", q=QB)
                        s3 = spair.rearrange("p (u q) -> p u q", q=QB)
                        u3 = u2_sb[:].rearrange("p (u q) -> p u q", q=QB)
                        nc.vector.scalar_tensor_tensor(
                            e3[:, :, so:QB], s3[:, :, so:QB], EXP_A / 8.0,
                            u3[:, :, 0:QB - so], ALU.mult, ALU.add)
                if debug and (qj, ki) == (0, 0):
                    nc.sync.dma_start(out=dbg["de"][:], in_=e[:])
                if debug and (qj, ki) == (1, 0):
                    nc.sync.dma_start(out=dbg["de2"][:], in_=e[:])
                ebuf[(qj, ki)] = (e, so)

            def emit_AV(qj, ki):
                e, so = ebuf.pop((qj, ki))
                last = ki == 4 * qj + 3
                for u in (0, 1):
                    nc.tensor.matmul(
                        arena[0:HD + 1, ycols(qj, u)][:, so:QB],
                        va_sb[ki][u][:],
                        e[:, u * QB + so:(u + 1) * QB],
                        start=(ki == 0), stop=last)

            def emit_DCOPY(qj):
                # d rows land on partitions 0 and 32 (legal AP bases).
                dd = sp.tile([33, QB], bf16, tag="dd", name=f"dd{qj}",
                             bufs=2)
                nc.scalar.copy(dd[0:1, :], arena[HD:HD + 1, ycols(qj, 0)])
                nc.vector.tensor_copy(dd[32:33, :],
                                      arena[HD:HD + 1, ycols(qj, 1)])
                if debug and qj == 0:
                    nc.sync.dma_start(out=dbg["ddd"][:], in_=dd[:])
                return dd

            def emit_CHAIN(qj, dd):
                # -1/d in bf16: magic seed + one Newton step.
                # r0 = bitcast(magic - bits(d)); t = d*r0; r1n = (t-2)*r0
                r0 = sp.tile([33, QB], bf16, tag="r0", name=f"r0_{qj}",
                             bufs=2)
                nc.vector.tensor_tensor(r0[:].bitcast(i16), mg_sb[:],
                                        dd[:].bitcast(i16), ALU.subtract)
                t = sp.tile([33, QB], bf16, tag="rt", name=f"rt{qj}",
                            bufs=2)
                nc.vector.tensor_mul(t[:], dd[:], r0[:])
                r1n = sp.tile([33, QB], bf16, tag="r1", name=f"r1_{qj}",
                              bufs=2)
                nc.vector.scalar_tensor_tensor(
                    r1n[:], t[:], 2.0, r0[:], ALU.subtract, ALU.mult)
                if debug and qj == 0:
                    nc.sync.dma_start(out=dbg["dr1"][:], in_=r1n[:])
                r1_t[qj] = r1n

            def emit_DP(qj):
                # broadcast -1/d across partitions with two K=1 matmuls
                # into the free bank of this chunk's (freed) yps pair.
                r1n = r1_t.pop(qj)
                dp = arena[0:P, YCOL[qj % 2] + QB:YCOL[qj % 2] + 2 * QB]
                nc.tensor.matmul(dp[0:HD, :], pm_sb[0:1, :], r1n[0:1, :],
                                 start=True, stop=True)
                nc.tensor.matmul(dp[HD:P, :], pm_sb[32:33, :],
                                 r1n[32:33, :], start=True, stop=True)
                return dp

            def emit_YRAW(qj):
                # evacuate unnormalized y early: frees the yps psum pair
                # without waiting for the reciprocal round-trip.
                yraw = sp.tile([P, QB], bf16, tag="yw", name=f"yw{qj}",
                               bufs=2)
                nc.scalar.copy(yraw[0:HD, :], arena[0:HD, ycols(qj, 0)])
                nc.vector.tensor_copy(yraw[HD:P, :],
                                      arena[0:HD, ycols(qj, 1)])
                return yraw

            def emit_YMUL(qj, yraw, dp):
                ysb = sp.tile([P, QB], bf16, tag="y", name=f"ysb{qj}",
                              bufs=2)
                nc.vector.tensor_mul(ysb[:], yraw[:], dp)
                if debug and qj == 0:
                    nc.sync.dma_start(out=dbg["dysb"][:], in_=ysb[:])
                ysb_t[qj] = ysb

            def emit_OP(qj, j, sfree=False):
                # out-proj c-chunk j; psum reuses the freed yps pair of
                # parity qj%2, or all four S banks in the epilogue.
                if sfree:
                    base = j * QB
                else:
                    base = YCOL[qj % 2] + (j % 2) * QB
                ops = arena[0:P, base:base + QB]
                nc.tensor.matmul(ops, wpT_sb[:, j * P:(j + 1) * P],
                                 ysb_t[qj][:], start=True, stop=True)

            def emit_OT(qj, pair, sfree=False):
                # evacuate op psum pair (2j, 2j+1) as one [128,1024] copy
                base = (2 * QB * pair) if sfree else YCOL[qj % 2]
                src = arena[0:P, base:base + 2 * QB]
                ot = sp.tile([P, 2 * QB], bf16, tag="ot",
                             name=f"ot{qj}_{pair}", bufs=2)
                if pair == 0:
                    nc.scalar.copy(ot[:], src)
                else:
                    nc.vector.tensor_copy(ot[:], src)
                ot_t[(qj, pair)] = ot

            outT3 = outT.rearrange("(k p) t -> p k t", p=P)

            def emit_OD(qj, pair):
                ot = ot_t.pop((qj, pair))
                nc.sync.dma_start(
                    out=outT3[:, 2 * pair:2 * pair + 2,
                              qj * QB:(qj + 1) * QB],
                    in_=ot[:].rearrange("p (k t) -> p k t", t=QB))

            # exp engine policy: diagonal -> DVE (fused mask); off-diag
            # mostly ACT, every 8th to DVE for balance.
            od_counter = [0]

            def exp_engine(qj, ki):
                if ki >= 4 * qj:
                    return "dve"
                od_counter[0] += 1
                return "dve" if od_counter[0] % 8 == 0 else "act"

            # ---- software-pipelined schedule ----
            emit_VP(0)
            vps0 = arena[0:P, YCOL[0]:YCOL[0] + QB]
            for h in range(2):
                sl = slice(h * (QB // 2), (h + 1) * (QB // 2))
                if with_bias:
                    nc.scalar.activation(vT_sb[:, sl], vps0[:, sl],
                                         AF.Copy, bias=bvc_sb)
                else:
                    nc.scalar.copy(vT_sb[:, sl], vps0[:, sl])
                for b in range(2):
                    sb = slice(h * (QB // 2) + b * P,
                               h * (QB // 2) + (b + 1) * P)
                    nc.gpsimd.tensor_mul(wvT_sb[:, sb], vT_sb[:, sb],
                                         wrp_sb[:, sb])
            emit_TR(0)

            def boundary_extras(pq, qj):
                """Ordered (slot, thunk) list: recip/norm/out-proj for
                chunk pq, and V-path prefetch for chunk qj+1."""
                st = {}
                ex = [
                    (1, lambda: st.__setitem__("dd", emit_DCOPY(pq))),
                    (2, lambda: emit_CHAIN(pq, st["dd"])),
                    (3, lambda: st.__setitem__("yw", emit_YRAW(pq))),
                ]
                if qj <= 2:
                    ex += [(4, lambda: emit_VP(qj + 1))]
                ex += [(4, lambda: st.__setitem__("dp", emit_DP(pq)))]
                if qj <= 2:
                    ex += [(5, lambda: (emit_VC(qj + 1), emit_WV(qj + 1)))]
                ex += [
                    (8, lambda: emit_YMUL(pq, st["yw"], st["dp"])),
                ]
                if qj <= 2:
                    ex += [(9, lambda: emit_TR(qj + 1))]
                ex += [
                    (10, lambda: emit_OP(pq, 0)),
                    (11, lambda: emit_OP(pq, 1)),
                    (12, lambda: (emit_OT(pq, 0), emit_OP(pq, 2))),
                    (13, lambda: (emit_OP(pq, 3), emit_OD(pq, 0))),
                    (14, lambda: emit_OT(pq, 1)),
                    (15, lambda: (emit_OD(pq, 1), ysb_t.pop(pq))),
                ]
                return ex

            for qj in range(NQ):
                nki = 4 * qj + 4
                if qj == 0:
                    extras = [(1, lambda: emit_VP(1)),
                              (2, lambda: (emit_VC(1), emit_WV(1))),
                              (3, lambda: emit_TR(1))]
                else:
                    extras = boundary_extras(qj - 1, qj)
                for i in range(nki + 2):
                    if i < nki:
                        spair, so = emit_QK(qj, i, i % 2)
                        emit_EXP(qj, i, spair, so, exp_engine(qj, i))
                    if 2 <= i <= nki + 1:
                        emit_AV(qj, i - 2)
                    while extras and extras[0][0] <= i:
                        extras.pop(0)[1]()
                for _, thunk in extras:
                    thunk()

            # ---- epilogue for the last chunk: all four S banks are
            # free, so the out-proj runs without OT-gating.
            pq = NQ - 1
            dd = emit_DCOPY(pq)
            emit_CHAIN(pq, dd)
            yw = emit_YRAW(pq)
            dp = emit_DP(pq)
            emit_YMUL(pq, yw, dp)
            for j in range(4):
                emit_OP(pq, j, sfree=True)
            emit_OT(pq, 0, sfree=True)
            emit_OT(pq, 1, sfree=True)
            emit_OD(pq, 0)
            emit_OD(pq, 1)
            ysb_t.pop(pq)
            if debug:
                nc.sync.dma_start(out=dbg["dvT"][:], in_=vT_sb[:])
                nc.sync.dma_start(out=dbg["dwvT"][:], in_=wvT_sb[:])
                dva3 = dbg["dva"].rearrange("p (k u c) -> p k u c",
                                            c=HD + 1, u=2)
                for kb in range(4):
                    for u in (0, 1):
                        nc.sync.dma_start(out=dva3[:, kb, u, :],
                                          in_=va_sb[kb][u][:])

    import concourse.mybir as mybir2
    _split_multi_waits(nc, mybir2)
    return nc


def _get_nc(with_bias=False, debug=False):
    key = f"nc{int(with_bias)}{int(debug)}"
    if key not in _cache:
        _cache[key] = _build_nc(with_bias, debug)
    return _cache[key]


def _make_in_maps(x, weight, Wv, bv, Wp, bp, state):
    x = np.asarray(x, np.float32)
    w = np.asarray(weight, np.float32)[:, :, 0]
    if not int(np.asarray(state)):
        w = np.ones_like(w)
    WvT = np.asarray(Wv, np.float32).T
    WpTn = -np.asarray(Wp, np.float32).T      # negated: folds -1/d sign
    bv = np.asarray(bv, np.float32)

    in_maps = []
    for core in range(8):
        b, hp = core // 4, core % 4
        js = slice(P * hp, P * (hp + 1))
        xTb = x[b].T.reshape(4, P, T).transpose(1, 0, 2).reshape(P, 4 * T)
        wvpb = WvT[:, js].reshape(4, P, P).transpose(1, 0, 2).reshape(P, C)
        wrpb = np.broadcast_to(w[b][None, :], (P, T))
        smfb = bv[js].reshape(P, 1)
        in_maps.append({
            "idn": np.eye(P, dtype=np.float32).astype(ml_dtypes.bfloat16),
            "xTp": np.ascontiguousarray(xTb).astype(ml_dtypes.bfloat16),
            "wvp": np.ascontiguousarray(wvpb).astype(ml_dtypes.bfloat16),
            "wpT": np.ascontiguousarray(WpTn[js, :]).astype(
                ml_dtypes.bfloat16),
            "wrp": np.ascontiguousarray(wrpb).astype(ml_dtypes.bfloat16),
            "smf": np.ascontiguousarray(smfb),
        })
    return in_maps


def _gather(results, x=None, bp=None):
    out = np.empty((B, T, C), np.float32)
    for b in range(B):
        acc = np.zeros((C, T), np.float32)
        for hp in range(4):
            acc += results[4 * b + hp]["outT"].astype(np.float32)
        out[b] = acc.T
    if bp is not None:
        out += np.asarray(bp, np.float32)[None, None, :]
    return out


def _run(in_maps, with_bias=False, debug=False, **kw):
    from concourse.bass_utils import run_bass_kernel_spmd
    return run_bass_kernel_spmd(
        _get_nc(with_bias, debug), in_maps, list(range(8)), **kw)


def kernel(x, weight, Wv, bv, Wp, bp, state):
    in_maps = _make_in_maps(x, weight, Wv, bv, Wp, bp, state)
    res = _run(in_maps, with_bias=bool(np.any(np.asarray(bv))))
    return _gather(res.results, x, bp)


# revision 23
# speedup vs baseline: 1.3730x; 1.0599x over previous
"""Causal self-attention (weight-modulated) Trainium2 kernel, 8-core SPMD.

Reference semantics (B=2, T=2048, C=512, 8 heads, hd=64):
    v0  = x @ Wv.T + bv
    att = softmax(mask((v0h @ v0h^T) * w[key] / sqrt(hd)))
    y   = att @ (v0*w[row])h
    out = y @ Wp.T + bp

Sharding: core = (b, hp) with b = batch, hp = head pair (v0 dims
[128hp, 128hp+128)).  v2 design notes:
  - w[key] is folded into wvT = vT * wrep (GpSimd), so QK against wvT
    yields pre-scaled scores and va = DMA-transpose(wvT) needs no
    further scaling.
  - exp is split across ACT (exact, off-diag blocks) and DVE
    (Schraudolph bf16 bit-trick: i16 = S*A + bias, bitcast; the causal
    mask of diagonal blocks is fused via scalar_tensor_tensor with a
    bias/mask tile: masked lanes get +4000 -> bitcast ~1e-25 ~ 0).
  - QK row-tiled: the two heads' K=64 matmuls run concurrently on
    array row strips (tile_position (0,0)/(64,0)).
  - softmax denominator rides as a 65th ones-column in the AV
    stationary; 1/d via bf16 magic-seed + 1 Newton step on DVE; the
    (negated) reciprocal is partition-broadcast by GpSimd and the sign
    is folded into a host-negated Wp.
  - PSUM is hand-placed in one [128, 4096] arena: banks 0-3 = S
    ping/pong, banks 4-7 = two yps pairs (alternating qj parity), with
    V-proj and out-proj matmuls reusing freed yps banks.
Host: out[b] = sum_hp outT^T + bp  (partial-sum reduce off-device).
"""

import ml_dtypes
import numpy as np

B, T, C = 2, 2048, 512
NH, HD = 8, 64
P = 128
QB = 512                 # query chunk
NQ = 4                   # query chunks
NKB = 16                 # key blocks of 128

EXP_A = 128 * 1.4426950408889634     # ln->bf16-exponent scale
EXP_BIAS = 16250.625                 # 16256 - C (C calibrated)
MASK_BIAS = 4000.0                   # masked lanes -> tiny positive
MAGIC16 = 0x7EF3                     # bf16 reciprocal seed

_cache = {}


def _split_multi_waits(nc, mybir):
    """Walrus in this container encodes at most ONE sync wait (and one
    update) per instruction; Tile's sem assignment emits several. Hoist
    excess waits onto single-wait NOPs placed just before the
    instruction on the same engine, and excess updates of non-DMA
    instructions onto NOPs just after."""
    dma_ops = {"DMACopy", "DMATranspose", "TensorCopy"}
    for f in nc.m.functions:
        for bb in f.blocks:
            new = []
            changed = False
            for inst in bb.instructions:
                si = inst.sync_info
                waits = list(si.on_wait or []) if si is not None else []
                ups = list(si.on_update or []) if si is not None else []
                is_dma = inst.concise_opcode() in dma_ops if hasattr(
                    inst, "concise_opcode") else False
                post = []
                if si is not None and len(waits) > 1:
                    for w in waits[:-1]:
                        nop = mybir.InstNoOp(
                            name=nc.get_next_instruction_name(),
                            sync_info=mybir.SyncInfo(on_wait=[w], on_update=[]),
                            bass_nofuse=True,
                            engine=inst.engine,
                        )
                        nc.register_instruction(nop, overwrite=True)
                        new.append(nop)
                    waits = waits[-1:]
                    inst.sync_info = mybir.SyncInfo(on_wait=waits, on_update=ups)
                    changed = True
                if si is not None and len(ups) > 1 and not is_dma:
                    for u in ups[1:]:
                        nop = mybir.InstNoOp(
                            name=nc.get_next_instruction_name(),
                            sync_info=mybir.SyncInfo(on_wait=[], on_update=[u]),
                            bass_nofuse=True,
                            engine=inst.engine,
                        )
                        nc.register_instruction(nop, overwrite=True)
                        post.append(nop)
                    inst.sync_info = mybir.SyncInfo(
                        on_wait=waits, on_update=ups[:1])
                    changed = True
                new.append(inst)
                new.extend(post)
            if changed:
                bb.instructions = new


def _u2bias():
    # U2[k, j] = EXP_BIAS if j >= k else MASK_BIAS, doubled side by side
    # so a [p, 2, span] strided AP serves both heads of a diagonal block.
    s = np.arange(P)[:, None]
    j = np.arange(QB)[None, :]
    u = np.where(j >= s, EXP_BIAS, MASK_BIAS).astype(np.float32)
    return np.concatenate([u, u], axis=1)


def _build_nc(with_bias, debug=False):
    import concourse.bass as bass
    import concourse.mybir as mybir
    from concourse.tile import TileContext

    f32 = mybir.dt.float32
    bf16 = mybir.dt.bfloat16
    i16 = mybir.dt.int16
    AF = mybir.ActivationFunctionType
    ALU = mybir.AluOpType

    nc = bass.Bass()

    xTp = nc.dram_tensor("xTp", [P, 4 * T], bf16, kind="ExternalInput")
    wvp = nc.dram_tensor("wvp", [P, C], bf16, kind="ExternalInput")
    wpT = nc.dram_tensor("wpT", [P, C], bf16, kind="ExternalInput")
    wrp = nc.dram_tensor("wrp", [P, T], bf16, kind="ExternalInput")
    smf = nc.dram_tensor("smf", [P, 1], f32, kind="ExternalInput")
    outT = nc.dram_tensor("outT", [C, T], bf16, kind="ExternalOutput")
    if debug:
        dbg = {
            "dvT": nc.dram_tensor("dvT", [P, T], bf16,
                                  kind="ExternalOutput"),
            "dwvT": nc.dram_tensor("dwvT", [P, T], bf16,
                                   kind="ExternalOutput"),
            "dva": nc.dram_tensor("dva", [P, 4 * (2 * HD + 2)], bf16,
                                  kind="ExternalOutput"),
            "de": nc.dram_tensor("de", [P, 2 * QB], bf16,
                                 kind="ExternalOutput"),
            "de2": nc.dram_tensor("de2", [P, 2 * QB], bf16,
                                  kind="ExternalOutput"),
            "ddd": nc.dram_tensor("ddd", [33, QB], bf16,
                                  kind="ExternalOutput"),
            "dr1": nc.dram_tensor("dr1", [33, QB], bf16,
                                  kind="ExternalOutput"),
            "drr": nc.dram_tensor("drr", [P, QB], bf16,
                                  kind="ExternalOutput"),
            "dysb": nc.dram_tensor("dysb", [P, QB], bf16,
                                   kind="ExternalOutput"),
        }

    idn = nc.dram_tensor("idn", [P, P], bf16, kind="ExternalInput")
    u2_d = nc.inline_tensor(_u2bias(), name="u2bias")
    mg_d = nc.inline_tensor(
        np.full((33, QB), MAGIC16, np.int16), name="magic16")
    pm_d = nc.inline_tensor(
        np.ones((33, HD), ml_dtypes.bfloat16), name="pmones")

    arena = nc.alloc_psum_tensor("arena", [P, 4096], f32)

    # arena column layout (each 512-col slab = one PSUM bank)
    SCOL = (0, 1024)                 # S ping/pong, [128, 1024] each
    YCOL = (2048, 3072)              # yps pair per qj parity

    with TileContext(nc) as tc:
        with (
            tc.tile_pool(name="persist", bufs=1) as pp,
            tc.tile_pool(name="stream", bufs=2) as sp,
        ):
            # ---- persistent SBUF ----
            xT_sb = pp.tile([P, 4 * T], bf16, tag="xTp")
            vT_sb = pp.tile([P, T], bf16, tag="vT")
            wvT_sb = pp.tile([P, T], bf16, tag="wvT")
            wrp_sb = pp.tile([P, T], bf16, tag="wrp")
            wv_sb = pp.tile([P, C], bf16, tag="wvp")
            wpT_sb = pp.tile([P, C], bf16, tag="wp")
            va_sb = [[pp.tile([P, HD + 1], bf16, tag=f"va{i}_{u}",
                              name=f"va{i}_{u}") for u in (0, 1)]
                     for i in range(NKB)]
            u2_sb = pp.tile([P, 2 * QB], f32, tag="u2")
            idn_sb = pp.tile([P, P], bf16, tag="idn")
            mg_sb = pp.tile([33, QB], i16, tag="mg")
            pm_sb = pp.tile([33, HD], bf16, tag="pm")
            smf_sb = pp.tile([P, 1], f32, tag="smf")
            bvc_sb = smf_sb[:, 0:1]

            # ---- prologue DMAs, split across sync + scalar queues ----
            xT3d = xTp.rearrange("p (k t) -> p k t", t=T)
            xT3s = xT_sb[:].rearrange("p (k t) -> p k t", t=T)
            nc.scalar.dma_start(out=idn_sb[:], in_=idn[:])
            for k in range(4):
                eng = nc.sync if k % 2 == 0 else nc.scalar
                eng.dma_start(out=xT3s[:, k, 0:QB], in_=xT3d[:, k, 0:QB])
            nc.sync.dma_start(out=wv_sb[:], in_=wvp[:])
            nc.scalar.dma_start(out=wrp_sb[:], in_=wrp[:])
            nc.sync.dma_start(out=u2_sb[:], in_=u2_d[:])
            for c in range(1, 4):
                eng = nc.sync if c % 2 == 1 else nc.scalar
                eng.dma_start(out=xT3s[:, :, c * QB:(c + 1) * QB],
                              in_=xT3d[:, :, c * QB:(c + 1) * QB])
            nc.scalar.dma_start(out=wpT_sb[:], in_=wpT[:])
            nc.sync.dma_start(out=mg_sb[:], in_=mg_d[:])
            nc.scalar.dma_start(out=pm_sb[:], in_=pm_d[:])
            if with_bias:
                nc.scalar.dma_start(out=smf_sb[:], in_=smf[:])
            for i in range(NKB):
                for u in (0, 1):
                    nc.gpsimd.memset(va_sb[i][u][:, HD:HD + 1], 1.0)

            # ---- warm-ups while the bulk DMAs land: dummy matmuls
            # hold the PE at the 2.4GHz clock; a tiny activation absorbs
            # the ~2.7us EXP table load before the first real exp.
            wact = sp.tile([1, 8], bf16, tag="wact", bufs=1)
            nc.scalar.activation(wact[:], idn_sb[0:1, 0:8], AF.Exp,
                                 scale=1.0)
            wps = arena[0:P, 0:P]
            for wi in range(24):
                nc.tensor.matmul(wps, idn_sb[:], idn_sb[:],
                                 start=(wi == 0), stop=(wi == 23))

            ebuf = {}
            ysb_t = {}
            r1_t = {}
            ot_t = {}

            def ycols(qj, u):
                base = YCOL[qj % 2]
                return slice(base + u * QB, base + (u + 1) * QB)

            def emit_VP(qj):
                vps = arena[0:P, YCOL[qj % 2]:YCOL[qj % 2] + QB]
                for k in range(4):
                    nc.tensor.matmul(
                        vps, wv_sb[:, k * P:(k + 1) * P],
                        xT_sb[:, k * T + qj * QB:k * T + (qj + 1) * QB],
                        start=(k == 0), stop=(k == 3))

            def emit_VC(qj):
                vps = arena[0:P, YCOL[qj % 2]:YCOL[qj % 2] + QB]
                dst = vT_sb[:, qj * QB:(qj + 1) * QB]
                if with_bias:
                    nc.scalar.activation(dst, vps, AF.Copy, bias=bvc_sb)
                else:
                    nc.scalar.copy(dst, vps)

            def emit_WV(qj):
                sl = slice(qj * QB, (qj + 1) * QB)
                nc.gpsimd.tensor_mul(wvT_sb[:, sl], vT_sb[:, sl],
                                     wrp_sb[:, sl])

            def emit_TR(qj):
                # PE-transpose each key block of wvT into transient bf16
                # psum (free half of chunk-qj's yps pair), then evacuate
                # per-head halves into the va tiles (ACT u0 / DVE u1).
                tbase = YCOL[qj % 2] + QB
                for g in range(4):
                    kb = 4 * qj + g
                    tps = arena[0:P, tbase + g * HD:
                                tbase + (g + 1) * HD].bitcast(bf16)
                    nc.tensor.transpose(
                        tps, wvT_sb[:, kb * P:(kb + 1) * P], idn_sb[:])
                    nc.vector.tensor_copy(va_sb[kb][0][:, 0:HD],
                                          tps[:, 0:HD])
                    nc.vector.tensor_copy(va_sb[kb][1][:, 0:HD],
                                          tps[:, HD:P])

            def emit_QK(qj, ki, sbuf_i):
                diag = ki >= 4 * qj
                so = P * (ki - 4 * qj) if diag else 0
                c0 = SCOL[sbuf_i]
                spair = arena[0:P, c0:c0 + 2 * QB]
                for u in (0, 1):
                    nc.tensor.matmul(
                        spair[:, u * QB + so:(u + 1) * QB],
                        wvT_sb[HD * u:HD * (u + 1), ki * P:(ki + 1) * P],
                        vT_sb[HD * u:HD * (u + 1),
                              qj * QB + so:(qj + 1) * QB],
                        start=True, stop=True,
                        tile_position=(HD * u, 0))
                return spair, so

            def emit_EXP(qj, ki, spair, so, eng):
                diag = ki >= 4 * qj
                e = sp.tile([P, 2 * QB], bf16, tag="e", name=f"e{qj}_{ki}",
                            bufs=6)
                if eng == "act":
                    if not diag:
                        nc.scalar.activation(e[:], spair, AF.Exp,
                                             scale=0.125)
                    else:
                        e3 = e[:].rearrange("p (u q) -> p u q", q=QB)
                        s3 = spair.rearrange("p (u q) -> p u q", q=QB)
                        nc.scalar.activation(
                            e3[:, :, so:QB], s3[:, :, so:QB], AF.Exp,
                            scale=0.125)
                        u3 = u2_sb[:].rearrange("p (u q) -> p u q", q=QB)
                        # mask on gpsimd: e *= (bias>=MASK? ) -- use
                        # 0/1 trick: compare not avail; multiply by U
                        # is handled below via DVE fallback; ACT-diag
                        # not used by default policy.
                        raise AssertionError("ACT diag not supported")
                else:
                    ei = e[:].bitcast(i16)
                    if not diag:
                        nc.vector.tensor_scalar(
                            ei, spair, EXP_A / 8.0, EXP_BIAS,
                            ALU.mult, ALU.add)
                    else:
                        e3 = ei.rearrange("p (u q) -> p u q", q=QB)
                        s3 = spair.rearrange("p (u q) -> p u q", q=QB)
                        u3 = u2_sb[:].rearrange("p (u q) -> p u q", q=QB)
                        nc.vector.scalar_tensor_tensor(
                            e3[:, :, so:QB], s3[:, :, so:QB], EXP_A / 8.0,
                            u3[:, :, 0:QB - so], ALU.mult, ALU.add)
                if debug and (qj, ki) == (0, 0):
                    nc.sync.dma_start(out=dbg["de"][:], in_=e[:])
                if debug and (qj, ki) == (1, 0):
                    nc.sync.dma_start(out=dbg["de2"][:], in_=e[:])
                ebuf[(qj, ki)] = (e, so)

            def emit_AV(qj, ki):
                e, so = ebuf.pop((qj, ki))
                last = ki == 4 * qj + 3
                for u in (0, 1):
                    nc.tensor.matmul(
                        arena[0:HD + 1, ycols(qj, u)][:, so:QB],
                        va_sb[ki][u][:],
                        e[:, u * QB + so:(u + 1) * QB],
                        start=(ki == 0), stop=last)

            def emit_DCOPY(qj):
                # d rows land on partitions 0 and 32 (legal AP bases).
                dd = sp.tile([33, QB], bf16, tag="dd", name=f"dd{qj}",
                             bufs=2)
                nc.scalar.copy(dd[0:1, :], arena[HD:HD + 1, ycols(qj, 0)])
                nc.vector.tensor_copy(dd[32:33, :],
                                      arena[HD:HD + 1, ycols(qj, 1)])
                if debug and qj == 0:
                    nc.sync.dma_start(out=dbg["ddd"][:], in_=dd[:])
                return dd

            def emit_CHAIN(qj, dd):
                # -1/d in bf16: magic seed + one Newton step.
                # r0 = bitcast(magic - bits(d)); t = d*r0; r1n = (t-2)*r0
                r0 = sp.tile([33, QB], bf16, tag="r0", name=f"r0_{qj}",
                             bufs=2)
                nc.vector.tensor_tensor(r0[:].bitcast(i16), mg_sb[:],
                                        dd[:].bitcast(i16), ALU.subtract)
                t = sp.tile([33, QB], bf16, tag="rt", name=f"rt{qj}",
                            bufs=2)
                nc.vector.tensor_mul(t[:], dd[:], r0[:])
                r1n = sp.tile([33, QB], bf16, tag="r1", name=f"r1_{qj}",
                              bufs=2)
                nc.vector.scalar_tensor_tensor(
                    r1n[:], t[:], 2.0, r0[:], ALU.subtract, ALU.mult)
                if debug and qj == 0:
                    nc.sync.dma_start(out=dbg["dr1"][:], in_=r1n[:])
                r1_t[qj] = r1n

            def emit_DP(qj):
                # broadcast -1/d across partitions with two K=1 matmuls
                # into the free bank of this chunk's (freed) yps pair.
                r1n = r1_t.pop(qj)
                dp = arena[0:P, YCOL[qj % 2] + QB:YCOL[qj % 2] + 2 * QB]
                for wi in range(2):
                    nc.tensor.matmul(dp, idn_sb[:], wrp_sb[:, 0:QB],
                                     start=(wi == 0), stop=(wi == 1))
                nc.tensor.matmul(dp[0:HD, :], pm_sb[0:1, :], r1n[0:1, :],
                                 start=True, stop=True)
                nc.tensor.matmul(dp[HD:P, :], pm_sb[32:33, :],
                                 r1n[32:33, :], start=True, stop=True)
                return dp

            def emit_YRAW(qj):
                # evacuate unnormalized y early: frees the yps psum pair
                # without waiting for the reciprocal round-trip.
                yraw = sp.tile([P, QB], bf16, tag="yw", name=f"yw{qj}",
                               bufs=2)
                nc.scalar.copy(yraw[0:HD, :], arena[0:HD, ycols(qj, 0)])
                nc.vector.tensor_copy(yraw[HD:P, :],
                                      arena[0:HD, ycols(qj, 1)])
                return yraw

            def emit_YMUL(qj, yraw, dp):
                ysb = sp.tile([P, QB], bf16, tag="y", name=f"ysb{qj}",
                              bufs=2)
                nc.vector.tensor_mul(ysb[:], yraw[:], dp)
                if debug and qj == 0:
                    nc.sync.dma_start(out=dbg["dysb"][:], in_=ysb[:])
                ysb_t[qj] = ysb

            def emit_OP(qj, j, sfree=False):
                # out-proj c-chunk j; psum reuses the freed yps pair of
                # parity qj%2, or all four S banks in the epilogue.
                if sfree:
                    base = j * QB
                else:
                    base = YCOL[qj % 2] + (j % 2) * QB
                ops = arena[0:P, base:base + QB]
                if j == 0 and not sfree:
                    for wi in range(2):
                        nc.tensor.matmul(ops, idn_sb[:], wrp_sb[:, 0:QB],
                                         start=(wi == 0), stop=(wi == 1))
                nc.tensor.matmul(ops, wpT_sb[:, j * P:(j + 1) * P],
                                 ysb_t[qj][:], start=True, stop=True)

            def emit_OT(qj, pair, sfree=False):
                # evacuate op psum pair (2j, 2j+1) as one [128,1024] copy
                base = (2 * QB * pair) if sfree else YCOL[qj % 2]
                src = arena[0:P, base:base + 2 * QB]
                ot = sp.tile([P, 2 * QB], bf16, tag="ot",
                             name=f"ot{qj}_{pair}", bufs=2)
                if pair == 0:
                    nc.scalar.copy(ot[:], src)
                else:
                    nc.vector.tensor_copy(ot[:], src)
                ot_t[(qj, pair)] = ot

            outT3 = outT.rearrange("(k p) t -> p k t", p=P)

            def emit_OD(qj, pair):
                ot = ot_t.pop((qj, pair))
                nc.sync.dma_start(
                    out=outT3[:, 2 * pair:2 * pair + 2,
                              qj * QB:(qj + 1) * QB],
                    in_=ot[:].rearrange("p (k t) -> p k t", t=QB))

            # exp engine policy: diagonal -> DVE (fused mask); off-diag
            # mostly ACT, every 8th to DVE for balance.
            od_counter = [0]

            def exp_engine(qj, ki):
                if ki >= 4 * qj:
                    return "dve"
                od_counter[0] += 1
                return "dve" if od_counter[0] % 8 == 0 else "act"

            # ---- software-pipelined schedule ----
            emit_VP(0)
            vps0 = arena[0:P, YCOL[0]:YCOL[0] + QB]
            for h in range(2):
                sl = slice(h * (QB // 2), (h + 1) * (QB // 2))
                if with_bias:
                    nc.scalar.activation(vT_sb[:, sl], vps0[:, sl],
                                         AF.Copy, bias=bvc_sb)
                else:
                    nc.scalar.copy(vT_sb[:, sl], vps0[:, sl])
                for b in range(2):
                    sb = slice(h * (QB // 2) + b * P,
                               h * (QB // 2) + (b + 1) * P)
                    nc.gpsimd.tensor_mul(wvT_sb[:, sb], vT_sb[:, sb],
                                         wrp_sb[:, sb])
            emit_TR(0)

            def boundary_extras(pq, qj):
                """Ordered (slot, thunk) list: recip/norm/out-proj for
                chunk pq, and V-path prefetch for chunk qj+1."""
                st = {}
                ex = [
                    (1, lambda: st.__setitem__("dd", emit_DCOPY(pq))),
                    (2, lambda: emit_CHAIN(pq, st["dd"])),
                    (3, lambda: st.__setitem__("yw", emit_YRAW(pq))),
                ]
                if qj <= 2:
                    ex += [(4, lambda: emit_VP(qj + 1))]
                ex += [(4, lambda: st.__setitem__("dp", emit_DP(pq)))]
                if qj <= 2:
                    ex += [(5, lambda: (emit_VC(qj + 1), emit_WV(qj + 1)))]
                ex += [
                    (8, lambda: emit_YMUL(pq, st["yw"], st["dp"])),
                ]
                if qj <= 2:
                    ex += [(9, lambda: emit_TR(qj + 1))]
                ex += [
                    (10, lambda: emit_OP(pq, 0)),
                    (11, lambda: emit_OP(pq, 1)),
                    (12, lambda: (emit_OT(pq, 0), emit_OP(pq, 2))),
                    (13, lambda: (emit_OP(pq, 3), emit_OD(pq, 0))),
                    (14, lambda: emit_OT(pq, 1)),
                    (15, lambda: (emit_OD(pq, 1), ysb_t.pop(pq))),
                ]
                return ex

            for qj in range(NQ):
                nki = 4 * qj + 4
                if qj == 0:
                    extras = [(1, lambda: emit_VP(1)),
                              (2, lambda: (emit_VC(1), emit_WV(1))),
                              (3, lambda: emit_TR(1))]
                else:
                    extras = boundary_extras(qj - 1, qj)
                for i in range(nki + 2):
                    if i < nki:
                        spair, so = emit_QK(qj, i, i % 2)
                        emit_EXP(qj, i, spair, so, exp_engine(qj, i))
                    if 2 <= i <= nki + 1:
                        emit_AV(qj, i - 2)
                    while extras and extras[0][0] <= i:
                        extras.pop(0)[1]()
                for _, thunk in extras:
                    thunk()

            # ---- epilogue for the last chunk: all four S banks are
            # free, so the out-proj runs without OT-gating.
            pq = NQ - 1
            dd = emit_DCOPY(pq)
            emit_CHAIN(pq, dd)
            yw = emit_YRAW(pq)
            dp = emit_DP(pq)
            emit_YMUL(pq, yw, dp)
            for j in range(4):
                emit_OP(pq, j, sfree=True)
            ot0 = sp.tile([P, 2 * QB], bf16, tag="ot", name="ote0",
                          bufs=2)
            ot1 = sp.tile([P, 2 * QB], bf16, tag="ot", name="ote1",
                          bufs=2)
            nc.scalar.copy(ot0[:, 0:QB], arena[0:P, 0:QB])
            nc.vector.tensor_copy(ot0[:, QB:2 * QB], arena[0:P, QB:2 * QB])
            nc.scalar.copy(ot1[:, 0:QB], arena[0:P, 2 * QB:3 * QB])
            nc.vector.tensor_copy(ot1[:, QB:2 * QB],
                                  arena[0:P, 3 * QB:4 * QB])
            ot_t[(pq, 0)] = ot0
            ot_t[(pq, 1)] = ot1
            emit_OD(pq, 0)
            emit_OD(pq, 1)
            ysb_t.pop(pq)
            if debug:
                nc.sync.dma_start(out=dbg["dvT"][:], in_=vT_sb[:])
                nc.sync.dma_start(out=dbg["dwvT"][:], in_=wvT_sb[:])
                dva3 = dbg["dva"].rearrange("p (k u c) -> p k u c",
                                            c=HD + 1, u=2)
                for kb in range(4):
                    for u in (0, 1):
                        nc.sync.dma_start(out=dva3[:, kb, u, :],
                                          in_=va_sb[kb][u][:])

    import concourse.mybir as mybir2
    _split_multi_waits(nc, mybir2)
    return nc


def _get_nc(with_bias=False, debug=False):
    key = f"nc{int(with_bias)}{int(debug)}"
    if key not in _cache:
        _cache[key] = _build_nc(with_bias, debug)
    return _cache[key]


def _make_in_maps(x, weight, Wv, bv, Wp, bp, state):
    x = np.asarray(x, np.float32)
    w = np.asarray(weight, np.float32)[:, :, 0]
    if not int(np.asarray(state)):
        w = np.ones_like(w)
    WvT = np.asarray(Wv, np.float32).T
    WpTn = -np.asarray(Wp, np.float32).T      # negated: folds -1/d sign
    bv = np.asarray(bv, np.float32)

    in_maps = []
    for core in range(8):
        b, hp = core // 4, core % 4
        js = slice(P * hp, P * (hp + 1))
        xTb = x[b].T.reshape(4, P, T).transpose(1, 0, 2).reshape(P, 4 * T)
        wvpb = WvT[:, js].reshape(4, P, P).transpose(1, 0, 2).reshape(P, C)
        wrpb = np.broadcast_to(w[b][None, :], (P, T))
        smfb = bv[js].reshape(P, 1)
        in_maps.append({
            "idn": np.eye(P, dtype=np.float32).astype(ml_dtypes.bfloat16),
            "xTp": np.ascontiguousarray(xTb).astype(ml_dtypes.bfloat16),
            "wvp": np.ascontiguousarray(wvpb).astype(ml_dtypes.bfloat16),
            "wpT": np.ascontiguousarray(WpTn[js, :]).astype(
                ml_dtypes.bfloat16),
            "wrp": np.ascontiguousarray(wrpb).astype(ml_dtypes.bfloat16),
            "smf": np.ascontiguousarray(smfb),
        })
    return in_maps


def _gather(results, x=None, bp=None):
    out = np.empty((B, T, C), np.float32)
    for b in range(B):
        acc = np.zeros((C, T), np.float32)
        for hp in range(4):
            acc += results[4 * b + hp]["outT"].astype(np.float32)
        out[b] = acc.T
    if bp is not None:
        out += np.asarray(bp, np.float32)[None, None, :]
    return out


def _run(in_maps, with_bias=False, debug=False, **kw):
    from concourse.bass_utils import run_bass_kernel_spmd
    return run_bass_kernel_spmd(
        _get_nc(with_bias, debug), in_maps, list(range(8)), **kw)


def kernel(x, weight, Wv, bv, Wp, bp, state):
    in_maps = _make_in_maps(x, weight, Wv, bv, Wp, bp, state)
    res = _run(in_maps, with_bias=bool(np.any(np.asarray(bv))))
    return _gather(res.results, x, bp)
